# revision 1
# baseline (speedup 1.0000x reference)
"""Trainium2 Bass kernel for nn_MoEClassifier (moe_routing).

Model (per sample):
  x[16,5] -> flat 80 -> fc1(80->64) gelu -> fc2(64->64) gelu -> LN -> h
  u = user_table[user_id]  (16)
  gate: g_e = sum_r (h @ gU[e])_r * (u @ gV[e])_r + gb_e ; top-2 softmax -> w
  experts (dense): z_e = gelu(h @ e_w1[e] + e_b1[e]); LN(z); lpe = z @ e_w2[e] + e_b2
  logits = sum_e w_e * lpe_e   (10 classes)

Strategy: pure data-parallel across 8 NeuronCores (batch 131072 -> 16384/core).
Feature-major on-chip layout ([feature partitions, batch free]).

Precision split (the gate's top-2 selection is tie-sensitive: min |g2-g3| gap
on this input set is ~3e-7, and bf16-level gate noise flips >1000 samples):
  - f32 path: backbone fc1/fc2, bb-LN stats + Newton rsqrt, gate A-matmul.
  - bf16 path (4x cheaper matmuls in cycles/row): everything expert-side
    (fc1/fc2/LN-stats/combine) plus the h broadcast used only by experts.

Gate refactor: h = LN(h2)*g+beta folds through the bilinear gate as
  g_e = rs * A_e - (mu*rs) * B_e + D_e
  A_e = sum_r psU0_er * uV_er      (device: psU0 = (g.wgU)^T @ h2, f32)
  B[u,e] = sum_r (wgU^T g)_er uV[u,er],  D[u,e] = sum_r (wgU^T beta)_er uV[u,er] + gb_e
(B, D, uV are host-precomputed per-user tables — weight prep, like prep_consts.)

Expert-LN fold (as baseline): lpe = rs*((z*g)@w2 - mu*(g@w2)) + (beta@w2+b2);
logits = sum_e ws_e*A'_e - sum_e wsm_e*gw2_e + sum_e w_e*cst_e, ws=w*rs, wsm=w*rs*mu.
Expert mu AND m2=mean(z^2) stats ride in the fc2 PSUM banks: block rows
32j+{0,1} = mu (via extra we2 columns), rows 32j+{2,3} = m2 (2nd matmul of an
accumulation group contracting z^2); cls at rows 32j+4..24.  Stats go
batch-major via full-bank bf16 transposes + one strided extraction copy.

fp16 hi/lo split matmuls (1 cyc/row vs 4 for f32, ~2^-22 accurate, verified
0 top-2 flips on this input set): bb fc1 (x split on host), bb-LN stats and
the gate A-matmul (h2 split on device).

Execution: 14-phase software pipeline, one phase per tile per outer step,
oldest tile first, so every cross-engine dependency crosses a step boundary
and each engine's FIFO receives work in data-ready order.  PSUM: 8 banks =
2(bb) + 2(z) + 2(fc) + 2(shared small).
"""
import sys, os

for _p in ("/opt/trn_rl_repo",):
    if _p not in sys.path:
        sys.path.insert(0, _p)

import numpy as np
from contextlib import ExitStack

import concourse.bass as bass
import concourse.tile as tile
from concourse import bacc, mybir

F32 = mybir.dt.float32
BF16 = mybir.dt.bfloat16
FP16 = mybir.dt.float16
I32 = mybir.dt.int32
AF = mybir.ActivationFunctionType
ALU = mybir.AluOpType

B = 131072
NCORES = 8
B_CORE = B // NCORES
IN_F = 80
EMB = 64
UDIM = 16
E = 16
RANK = 8
NCLS = 10
NUSERS = 1000
EPS_LN = 1e-5
TN = 512
NCH = TN // 128


def _bc(ap, n):
    """broadcast the (size-1) innermost dim of an AP to n via stride 0"""
    return ap.to_broadcast(list(ap.shape[:-1]) + [n])


def _pslice(t, base, cnt):
    """partition slice [base:base+cnt] of tile t's full AP"""
    return t[base:base + cnt]



# packed constant layouts: name -> (partitions, col offset, col width)
CF32_OFF = {
    "identf": (128, 0, 128), "wbb1": (IN_F, 128, EMB), "wbb2": (EMB, 192, EMB),
    "b1": (EMB, 256, 1), "b2": (EMB, 257, 1), "beta": (EMB, 258, 1),
    "stat2": (128, 259, 2), "wgU0": (EMB, 261, 128), "gsum": (128, 389, E),
    "eb1": (128, 405, 8),
}
CF32_COLS = 413
CB16_OFF = {
    "identb": (128, 0, 128), "stlb": (2, 128, 128), "we1b": (EMB, 256, 1024),
    "we2b": (128, 1280, 256), "zwb": (128, 1536, 32), "wsbb": (48, 1568, 256),
    "msumb": (128, 1824, NCLS), "gw2cb": (2 * E, 1834, NCLS),
    "permg": (128, 1844, 32),
}
CB16_COLS = 1876


def build_program(b_core=B_CORE, mmdt="hybrid", bufs=None):
    ntiles = b_core // TN
    nc = bacc.Bacc("TRN2", target_bir_lowering=False, debug=False,
                   num_devices=NCORES)

    # ---------------- DRAM I/O ----------------
    d_x = nc.dram_tensor("x", [ntiles, IN_F, 2 * TN], FP16, kind="ExternalInput")
    d_u = nc.dram_tensor("u", [ntiles, 128, TN], F32, kind="ExternalInput")
    d_BD = nc.dram_tensor("BD", [ntiles, 128, NCH, 2 * E], F32, kind="ExternalInput")
    d_out = nc.dram_tensor("out", [ntiles, NCH, 128, NCLS], F32, kind="ExternalOutput")

    def cin(name, shape, dt=F32):
        return nc.dram_tensor(name, shape, dt, kind="ExternalInput")

    d_cf32 = cin("cf32", [128, CF32_COLS])
    d_cb16 = cin("cb16", [128, CB16_COLS], BF16)
    d_ch16 = cin("ch16", [128, 388], FP16)   # wbb1 h/l | wgU0 h/l | stat2 | zeros2

    bu = {"inp": 3, "work": 3, "scal": 3, "zsb": 9, "z2sb": 3, "osb": 3,
          "psbb": 2, "psz": 2, "psfc": 2, "pssm": 2}
    for k in list(bu):
        v = os.environ.get("KB_" + k)
        if v:
            bu[k] = int(v)
    if bufs:
        bu.update(bufs)

    with tile.TileContext(nc) as tc, ExitStack() as ctx:
        cpool = ctx.enter_context(tc.tile_pool(name="consts", bufs=1))
        p_in = ctx.enter_context(tc.tile_pool(name="inp", bufs=bu["inp"]))
        p_w = ctx.enter_context(tc.tile_pool(name="work", bufs=bu["work"]))
        p_sc = ctx.enter_context(tc.tile_pool(name="scal", bufs=bu["scal"]))
        p_z = ctx.enter_context(tc.tile_pool(name="zsb", bufs=bu["zsb"]))
        p_z2 = ctx.enter_context(tc.tile_pool(name="z2sb", bufs=bu["z2sb"]))
        p_out = ctx.enter_context(tc.tile_pool(name="osb", bufs=bu["osb"]))
        ps_bb = ctx.enter_context(tc.tile_pool(name="psbb", bufs=bu["psbb"], space="PSUM"))
        ps_z = ctx.enter_context(tc.tile_pool(name="psz", bufs=bu["psz"], space="PSUM"))
        ps_fc = ctx.enter_context(tc.tile_pool(name="psfc", bufs=bu["psfc"], space="PSUM"))
        ps_sm = ctx.enter_context(tc.tile_pool(name="pssm", bufs=bu["pssm"], space="PSUM"))

        # ------------- constants: two packed DMAs, sliced views -------------
        t32 = cpool.tile([128, CF32_COLS], F32, tag="cf32", name="c_f32")
        nc.sync.dma_start(t32[:], d_cf32.ap())
        t16 = cpool.tile([128, CB16_COLS], BF16, tag="cb16", name="c_b16")
        nc.sync.dma_start(t16[:], d_cb16.ap())
        c = {}
        for name, (p, o, w) in CF32_OFF.items():
            c[name] = t32[0:p, o:o + w]
        for name, (p, o, w) in CB16_OFF.items():
            c[name] = t16[0:p, o:o + w]
        t_h16 = cpool.tile([128, 388], FP16, tag="ch16", name="c_h16")
        nc.sync.dma_start(t_h16[:], d_ch16.ap())
        c["wbb1h"] = t_h16[0:IN_F, 0:EMB]
        c["wbb1l"] = t_h16[0:IN_F, EMB:2 * EMB]
        c["wgU0h"] = t_h16[0:EMB, 128:256]
        c["wgU0l"] = t_h16[0:EMB, 256:384]
        c["stat2h"] = t_h16[0:128, 384:386]
        c["zc2"] = t_h16[0:128, 386:388]
        c["we1b"] = c["we1b"].rearrange("p (a b) -> p a b", a=8, b=128)
        c["we2b"] = c["we2b"].rearrange("p (a b) -> p a b", a=8, b=32)
        c["wsbb"] = c["wsbb"].rearrange("p (a b) -> p a b", a=2, b=128)
        identf = c["identf"]
        identb = c["identb"]

        def tile_body(it):
            # ==== P0: input DMAs + backbone fc1 ====
            x_fm = p_in.tile([IN_F, 2 * TN], FP16, tag="x_fm", bufs=3, name=f"x_{it}")
            nc.sync.dma_start(x_fm[:], d_x.ap()[it])
            u_fm = p_in.tile([128, TN], F32, tag="u_fm", bufs=4, name=f"u_{it}")
            nc.sync.dma_start(u_fm[:], d_u.ap()[it])
            BD_t = p_in.tile([128, NCH, 2 * E], F32, tag="BD", bufs=7, name=f"BD_{it}")
            nc.sync.dma_start(BD_t[:], d_BD.ap()[it])
            BT_t = BD_t[:, :, 0:E]
            DT_t = BD_t[:, :, E:2 * E]

            ps1 = ps_bb.tile([EMB, TN], F32, tag="bb", name=f"ps1_{it}")
            nc.tensor.matmul(ps1[:], c["wbb1h"], x_fm[:, 0:TN], start=True, stop=False)
            nc.tensor.matmul(ps1[:], c["wbb1h"], x_fm[:, TN:2 * TN], start=False, stop=False)
            nc.tensor.matmul(ps1[:], c["wbb1l"], x_fm[:, 0:TN], start=False, stop=True)
            h1 = p_w.tile([EMB, TN], F32, tag="h1", bufs=3, name=f"h1_{it}")
            nc.scalar.activation(h1[:], ps1[:], AF.Gelu, bias=c["b1"])

            yield  # ==== P1: backbone fc2 + square ====
            ps2 = ps_bb.tile([EMB, TN], F32, tag="bb", name=f"ps2_{it}")
            nc.tensor.matmul(ps2[:], c["wbb2"], h1[:], start=True, stop=True)
            h2s = p_w.tile([128, TN], F32, tag="h2s", bufs=6, name=f"h2s_{it}")
            nc.scalar.activation(h2s[0:EMB, :], ps2[:], AF.Gelu, bias=c["b2"])
            nc.vector.tensor_tensor(h2s[EMB:128, :], h2s[0:EMB, :], h2s[0:EMB, :],
                                    op=ALU.mult)
            h2hi = p_w.tile([128, TN], FP16, tag="h2hi", bufs=3, name=f"h2hi_{it}")
            nc.gpsimd.tensor_tensor(h2hi[:], h2s[:], _bc(c["zc2"][:, 0:1], TN),
                                    op=ALU.add)
            h2lo = p_w.tile([128, TN], FP16, tag="h2lo", bufs=3, name=f"h2lo_{it}")
            nc.gpsimd.tensor_tensor(h2lo[:], h2s[:], h2hi[:], op=ALU.subtract)

            yield  # ==== P2: bb-LN stats + gate A matmul ====
            psb = ps_sm.tile([2, TN], F32, tag="sm", name=f"psb_{it}")
            nc.tensor.matmul(psb[:], c["stat2h"], h2hi[:], start=True, stop=False)
            nc.tensor.matmul(psb[:], c["stat2h"], h2lo[:], start=False, stop=True)
            stats_bb = p_sc.tile([2, TN], F32, tag="stats_bb", bufs=3, name=f"sbb_{it}")
            nc.scalar.copy(stats_bb[:], psb[:])
            psU0 = ps_bb.tile([128, TN], F32, tag="bb", name=f"psU0_{it}")
            nc.tensor.matmul(psU0[:], c["wgU0h"], h2hi[0:EMB, :], start=True, stop=False)
            nc.tensor.matmul(psU0[:], c["wgU0h"], h2lo[0:EMB, :], start=False, stop=False)
            nc.tensor.matmul(psU0[:], c["wgU0l"], h2hi[0:EMB, :], start=False, stop=True)
            gprod = p_w.tile([128, TN], F32, tag="gprod", bufs=4, name=f"gprod_{it}")
            nc.vector.tensor_tensor(gprod[:], psU0[:], u_fm[:], op=ALU.mult)

            yield  # ==== P3: pass A (bb LN scalars) + gate sum ====
            psA = ps_sm.tile([128, NCH, 2], F32, tag="sm", name=f"psA_{it}")
            for ch in range(NCH):
                nc.tensor.transpose(psA[:, ch, :], stats_bb[:, 128 * ch:128 * (ch + 1)],
                                    identf[0:2, 0:2])
            sA = p_sc.tile([128, NCH, 2], F32, tag="sA", bufs=3, name=f"sA_{it}")
            nc.scalar.copy(sA[:], psA[:])
            tmpA = p_sc.tile([128, NCH], F32, tag="tmpA", bufs=3, name=f"tmpA_{it}")
            nc.vector.tensor_tensor(tmpA[:], sA[:, :, 0], sA[:, :, 0], op=ALU.mult)
            vA = p_sc.tile([128, NCH], F32, tag="vA", bufs=3, name=f"vA_{it}")
            nc.vector.scalar_tensor_tensor(vA[:], sA[:, :, 1], EPS_LN, tmpA[:],
                                           op0=ALU.add, op1=ALU.subtract)
            backA = p_sc.tile([128, NCH, 2], F32, tag="backA", bufs=4, name=f"backA_{it}")
            rsA = backA[:, :, 0]
            _newton_rsqrt(nc, p_sc, vA[:], rsA, [128, NCH], f"nA_{it}", niter=2)
            nc.vector.tensor_tensor(backA[:, :, 1], rsA, sA[:, :, 0], op=ALU.mult)
            backAb = p_sc.tile([128, NCH, 2], BF16, tag="backAb", bufs=3, name=f"backAb_{it}")
            nc.vector.tensor_copy(backAb[:], backA[:])

            psg = ps_bb.tile([E, TN], F32, tag="bb", name=f"psg_{it}")
            nc.tensor.matmul(psg[:], c["gsum"], gprod[:], start=True, stop=True)
            A_sb = p_sc.tile([E, TN], F32, tag="A_sb", bufs=4, name=f"Asb_{it}")
            nc.scalar.copy(A_sb[:], psg[:])

            yield  # ==== P4: rs/p broadcast transpose ====
            psBA = ps_bb.tile([2, TN], BF16, tag="bb", name=f"psBA_{it}")
            for ch in range(NCH):
                nc.tensor.transpose(psBA[:, 128 * ch:128 * (ch + 1)],
                                    backAb[:, ch, :], identb)
            stf = p_sc.tile([2, TN], BF16, tag="stf", bufs=3, name=f"stf_{it}")
            nc.scalar.copy(stf[:], psBA[:])

            yield  # ==== P5: h for experts + batch-major gate ====
            stp = ps_sm.tile([128, TN], F32, tag="sm", name=f"stp_{it}")
            nc.tensor.matmul(stp[:], c["stlb"], stf[:], start=True, stop=True)
            t1h = p_w.tile([EMB, TN], BF16, tag="t1h", bufs=3, name=f"t1h_{it}")
            nc.vector.tensor_tensor(t1h[:], h2s[0:EMB, :], stp[0:EMB, :], op=ALU.mult)
            hb = p_w.tile([EMB, TN], BF16, tag="hb", bufs=4, name=f"hb_{it}")
            nc.vector.scalar_tensor_tensor(hb[:], t1h[:], c["beta"],
                                           stp[EMB:128, :], op0=ALU.add, op1=ALU.add)

            psAT = ps_sm.tile([128, NCH, E], F32, tag="sm", name=f"psAT_{it}")
            for ch in range(NCH):
                nc.tensor.transpose(psAT[:, ch, :], A_sb[:, 128 * ch:128 * (ch + 1)],
                                    identf[0:E, 0:E])
            ATc = p_sc.tile([128, NCH, E], F32, tag="ATc", bufs=3, name=f"ATc_{it}")
            nc.scalar.copy(ATc[:], psAT[:])
            # g = rs*A - p*B + D    (batch-major, f32)
            g1t = p_sc.tile([128, NCH, E], F32, tag="g1t", bufs=3, name=f"g1t_{it}")
            nc.vector.tensor_tensor(g1t[:], ATc[:], _bc(backA[:, :, 0:1], E), op=ALU.mult)
            g2t = p_sc.tile([128, NCH, E], F32, tag="g2t", bufs=3, name=f"g2t_{it}")
            nc.vector.tensor_tensor(g2t[:], BT_t, _bc(backA[:, :, 1:2], E), op=ALU.mult)
            g3t = p_sc.tile([128, NCH, E], F32, tag="g3t", bufs=3, name=f"g3t_{it}")
            nc.vector.tensor_tensor(g3t[:], g1t[:], g2t[:], op=ALU.subtract)
            gcp = p_sc.tile([128, NCH, E], F32, tag="gcp", bufs=6, name=f"gcp_{it}")
            nc.vector.tensor_tensor(gcp[:], g3t[:], DT_t, op=ALU.add)

            yield  # ==== P6: experts fc1 + gelu + z^2 ====
            z_sb = []
            for p in range(8):
                zq = ps_z.tile([128, TN], F32, tag="z", name=f"zq_{it}_{p}")
                nc.tensor.matmul(zq[:], c["we1b"][:, p, :], hb[:], start=True, stop=True)
                z = p_z.tile([128, TN], BF16, tag="z_sb", bufs=18, name=f"z_{it}_{p}")
                nc.scalar.activation(z[:], zq[:], AF.Gelu, bias=c["eb1"][:, p:p + 1])
                z_sb.append(z)
            z2_sb = []
            for p in range(8):
                z2 = p_z2.tile([128, TN], BF16, tag="z2_sb", bufs=18, name=f"z2_{it}_{p}")
                eng = nc.gpsimd if p < int(os.environ.get('KZ2POOL', '0')) else nc.vector
                eng.tensor_tensor(z2[:], z_sb[p][:], z_sb[p][:], op=ALU.mult)
                z2_sb.append(z2)

            yield  # ==== P7: experts fc2 + LN stats (accumulation groups) ====
            fc = [ps_fc.tile([128, TN], F32, tag="fc", name=f"fc_{it}_{i}")
                  for i in range(2)]
            for grp in range(2):
                for j in range(4):
                    p = 4 * grp + j
                    nc.tensor.matmul(fc[grp][32 * j:32 * j + 32, :],
                                     c["we2b"][:, p, :], z_sb[p][:],
                                     start=True, stop=False, tile_position=(0, 32 * j))
                    nc.tensor.matmul(fc[grp][32 * j:32 * j + 32, :],
                                     c["zwb"], z2_sb[p][:],
                                     start=False, stop=True, tile_position=(0, 32 * j))
            fc2sb = []
            for b in range(2):
                t = p_w.tile([128, TN], BF16, tag="fc2sb", bufs=10, name=f"fc2sb_{it}_{b}")
                nc.scalar.copy(t[:], fc[b][:])
                fc2sb.append(t)

            yield  # ==== P8: stat transposes ====
            statB = p_sc.tile([128, NCH, 2, 4, 2, 2], F32, tag="statB", bufs=3,
                              name=f"statB_{it}")
            for b in range(2):
                psT = ps_sm.tile([128, NCH, 128], BF16, tag="sm", name=f"psT_{it}_{b}")
                for ch in range(NCH):
                    nc.tensor.transpose(psT[:, ch, :],
                                        fc2sb[b][:, 128 * ch:128 * (ch + 1)],
                                        identb)
                psT6 = psT[:].rearrange("p c (j v s q) -> p c j v s q",
                                        j=4, v=8, s=2, q=2)
                nc.vector.tensor_copy(statB[:, :, b], psT6[:, :, :, 0])
            muB = statB[:, :, :, :, 0, :]   # [128, NCH, 2, 4, 2] e-ordered (b,j,q)
            m2B = statB[:, :, :, :, 1, :]

            yield  # ==== P9: pass B (expert LN rs + top-2 gate) ====
            tmpB = p_sc.tile([128, NCH, 2, 4, 2], F32, tag="tmpB", bufs=3, name=f"tmpB_{it}")
            nc.gpsimd.tensor_tensor(tmpB[:], muB, muB, op=ALU.mult)
            vB = p_sc.tile([128, NCH, 2, 4, 2], F32, tag="vB", bufs=3, name=f"vB_{it}")
            nc.vector.scalar_tensor_tensor(vB[:], m2B, EPS_LN, tmpB[:],
                                           op0=ALU.add, op1=ALU.subtract)
            rsB = p_sc.tile([128, NCH, 2, 4, 2], F32, tag="rsB", bufs=3, name=f"rsB_{it}")
            _newton_rsqrt(nc, p_sc, vB[:], rsB[:], [128, NCH, 2, 4, 2],
                          f"nB_{it}", niter=1)

            vm8 = p_sc.tile([128, NCH, 8], F32, tag="vm8", bufs=3, name=f"vm8_{it}")
            for ch in range(NCH):
                nc.vector.max(vm8[:, ch, :], gcp[:, ch, :])
            dg = p_sc.tile([128, NCH], F32, tag="dg", bufs=3, name=f"dg_{it}")
            nc.vector.tensor_tensor(dg[:], vm8[:, :, 0], vm8[:, :, 1], op=ALU.subtract)
            th = p_sc.tile([128, NCH], F32, tag="th", bufs=3, name=f"th_{it}")
            nc.scalar.activation(th[:], dg[:], AF.Tanh, scale=0.5)
            w12 = p_sc.tile([128, NCH, 2], F32, tag="w12", bufs=3, name=f"w12_{it}")
            nc.vector.tensor_scalar(w12[:, :, 0], th[:], 0.5, 0.5, op0=ALU.mult, op1=ALU.add)
            nc.vector.tensor_scalar(w12[:, :, 1], th[:], -0.5, 0.5, op0=ALU.mult, op1=ALU.add)

            is1 = p_sc.tile([128, NCH, E], F32, tag="is1", bufs=3, name=f"is1_{it}")
            nc.vector.tensor_tensor(is1[:], gcp[:], _bc(vm8[:, :, 0:1], E), op=ALU.is_equal)
            is2 = p_sc.tile([128, NCH, E], F32, tag="is2", bufs=3, name=f"is2_{it}")
            nc.vector.tensor_tensor(is2[:], gcp[:], _bc(vm8[:, :, 1:2], E), op=ALU.is_equal)
            w1t = p_sc.tile([128, NCH, E], F32, tag="w1t", bufs=3, name=f"w1t_{it}")
            nc.vector.tensor_tensor(w1t[:], is1[:], _bc(w12[:, :, 0:1], E), op=ALU.mult)
            w2t = p_sc.tile([128, NCH, E], F32, tag="w2t", bufs=3, name=f"w2t_{it}")
            nc.vector.tensor_tensor(w2t[:], is2[:], _bc(w12[:, :, 1:2], E), op=ALU.mult)

            # back block (bf16): cols 0-15 wsm, 16-31 w, 32-47 ws, 48-63 pad
            backB = p_sc.tile([128, NCH, 64], BF16, tag="backB", bufs=4, name=f"backB_{it}")
            nc.gpsimd.memset(backB[:].rearrange("p c k -> p (c k)"), 0.0)
            nc.gpsimd.tensor_tensor(backB[:, :, 16:32], w1t[:], w2t[:], op=ALU.add)
            rsBf = rsB[:].rearrange("p c b j q -> p c (b j q)")
            nc.gpsimd.tensor_tensor(backB[:, :, 32:48], backB[:, :, 16:32], rsBf,
                                    op=ALU.mult)
            ws5 = backB[:, :, 32:48].rearrange("p c (b j q) -> p c b j q",
                                               b=2, j=4, q=2)
            wsm5 = backB[:, :, 0:16].rearrange("p c (b j q) -> p c b j q",
                                               b=2, j=4, q=2)
            nc.gpsimd.tensor_tensor(wsm5, ws5, muB, op=ALU.mult)

            yield  # ==== P10: weight-block transpose ====
            psBB = ps_sm.tile([128, 2, 128], BF16, tag="sm", name=f"psBB_{it}")
            backBv = backB[:].rearrange("p c k -> p (c k)")
            for hh in range(2):
                nc.tensor.transpose(psBB[:, hh, :],
                                    backBv[:, 128 * hh:128 * (hh + 1)], identb)
            cf = p_sc.tile([48, TN], BF16, tag="cf", bufs=4, name=f"cf_{it}")
            cfv = cf[:].rearrange("p (h c q) -> p h c q", h=2, c=2, q=128)
            nc.vector.tensor_copy(cfv[:, :, 0, :], psBB[0:48, :, :])
            nc.vector.tensor_copy(cfv[:, :, 1, :], psBB[64:112, :, :])

            yield  # ==== P11: ws broadcast + weighted fc2 ====
            prods = []
            for b in range(2):
                wsr = ps_sm.tile([128, TN], F32, tag="sm", name=f"wsr_{it}_{b}")
                nc.tensor.matmul(wsr[:], c["wsbb"][32:48, b, :], cf[32:48, :],
                                 start=True, stop=True)
                prod = p_w.tile([128, TN], BF16, tag="prod", bufs=6, name=f"prod_{it}_{b}")
                nc.vector.tensor_tensor(prod[:], fc2sb[b][:], wsr[:], op=ALU.mult)
                prods.append(prod)

            yield  # ==== P12: combine ====
            lg = ps_sm.tile([NCLS, TN], F32, tag="sm", name=f"lg_{it}")
            nc.tensor.matmul(lg[:], c["msumb"], prods[0][:], start=True, stop=False)
            nc.tensor.matmul(lg[:], c["msumb"], prods[1][:], start=False, stop=False)
            nc.tensor.matmul(lg[:], c["gw2cb"], cf[0:32, :], start=False, stop=True)
            lsb = p_out.tile([NCLS, TN], F32, tag="lsb", bufs=4, name=f"lsb_{it}")
            nc.scalar.copy(lsb[:], lg[:])

            yield  # ==== P13: output transpose + DMA ====
            psL = ps_bb.tile([128, NCH * NCLS], F32, tag="bb", name=f"psL_{it}")
            for ch in range(NCH):
                nc.tensor.transpose(psL[:, NCLS * ch:NCLS * (ch + 1)],
                                    lsb[:, 128 * ch:128 * (ch + 1)],
                                    identf[0:NCLS, 0:NCLS])
            osb = p_out.tile([128, NCH, NCLS], F32, tag="osb", bufs=3, name=f"osb_{it}")
            nc.vector.tensor_copy(osb[:], psL[:])
            nc.sync.dma_start(d_out.ap()[it].rearrange("c p k -> p c k"), osb[:])

        # 14-phase software pipeline, oldest tile first within each step so
        # slot-freeing work always precedes the allocations that reuse slots,
        # and every cross-engine dependency crosses a step boundary.
        NPH = 14
        gens = {}
        for k in range(ntiles + NPH - 1):
            if k < ntiles:
                gens[k] = tile_body(k)
            for idx in sorted(gens):
                if next(gens[idx], StopIteration) is StopIteration:
                    del gens[idx]

    nc.compile()
    return nc


def _newton_rsqrt(nc, pool, v_ap, out_ap, shape, tag, niter=2, eng=None):
    """out = 1/sqrt(v) via quake seed + Newton iterations."""
    eng = eng or nc.vector
    r = pool.tile(shape, F32, tag=tag[:3] + "_r", name=tag + "_r")
    t = pool.tile(shape, F32, tag=tag[:3] + "_t", name=tag + "_t")
    eng.tensor_scalar(r[:].bitcast(I32), v_ap.bitcast(I32), 1, None,
                      op0=ALU.logical_shift_right)
    eng.tensor_scalar(r[:].bitcast(I32), r[:].bitcast(I32), -1, 0x5F3759DF,
                      op0=ALU.mult, op1=ALU.add)
    for i in range(niter):
        dst = out_ap if i == niter - 1 else r[:]
        eng.tensor_tensor(t[:], r[:], r[:], op=ALU.mult)
        eng.scalar_tensor_tensor(t[:], t[:], -0.5, v_ap, op0=ALU.mult, op1=ALU.mult)
        eng.scalar_tensor_tensor(dst, t[:], 1.5, r[:], op0=ALU.add, op1=ALU.mult)


# ---------------------------------------------------------------------------
# host-side weight prep
# ---------------------------------------------------------------------------
def prep_consts(inp):
    f = np.float32
    import ml_dtypes
    bf = ml_dtypes.bfloat16
    gU = np.asarray(inp["gU"], np.float64)
    gb = np.asarray(inp["gb"], np.float64)
    e_w1, e_b1 = np.asarray(inp["e_w1"], f), np.asarray(inp["e_b1"], f)
    e_g, e_beta = np.asarray(inp["e_g"], f), np.asarray(inp["e_beta"], f)
    e_w2, e_b2 = np.asarray(inp["e_w2"], f), np.asarray(inp["e_b2"], f)
    bb_g = np.asarray(inp["bb_g"], np.float64)
    bb_beta = np.asarray(inp["bb_beta"], np.float64)

    vals32 = {}
    vals32["identf"] = np.eye(128, dtype=f)
    vals32["wbb1"] = np.asarray(inp["bb_w1"], f)
    vals32["wbb2"] = np.asarray(inp["bb_w2"], f)
    vals32["b1"] = np.asarray(inp["bb_b1"], f).reshape(EMB, 1)
    vals32["b2"] = np.asarray(inp["bb_b2"], f).reshape(EMB, 1)
    vals32["beta"] = bb_beta.astype(f).reshape(EMB, 1)
    st = np.zeros((128, 2), f)
    st[0:64, 0] = 1.0 / 64
    st[64:128, 1] = 1.0 / 64
    vals32["stat2"] = st
    wgU0 = np.zeros((EMB, 128), np.float64)
    for e in range(E):
        wgU0[:, e * RANK:(e + 1) * RANK] = gU[e] * bb_g[:, None]
    vals32["wgU0"] = wgU0.astype(f)
    _wgU0_f64 = wgU0
    gs = np.zeros((128, E), f)
    for e in range(E):
        gs[e * RANK:(e + 1) * RANK, e] = 1.0
    vals32["gsum"] = gs
    eb1 = np.zeros((128, 8), f)
    for p in range(8):
        eb1[0:64, p] = e_b1[2 * p]
        eb1[64:128, p] = e_b1[2 * p + 1]
    vals32["eb1"] = eb1

    vals16 = {}
    vals16["identb"] = np.eye(128, dtype=f)
    stl = np.zeros((2, 128), np.float64)
    stl[0, 0:64] = bb_g
    stl[1, 64:128] = -bb_g
    vals16["stlb"] = stl
    we1 = np.zeros((EMB, 8, 128), f)
    for p in range(8):
        we1[:, p, 0:64] = e_w1[2 * p]
        we1[:, p, 64:128] = e_w1[2 * p + 1]
    vals16["we1b"] = we1.reshape(EMB, 1024)
    # fc2 lhsT: cols 0/1 mu weights, 2/3 zero (m2 via zwb), 4:14 cls e0, 14:24 cls e1
    we2 = np.zeros((128, 8, 32), f)
    for p in range(8):
        e0, e1 = 2 * p, 2 * p + 1
        we2[0:64, p, 0] = 1.0 / 64
        we2[64:128, p, 1] = 1.0 / 64
        we2[0:64, p, 4:14] = e_g[e0][:, None] * e_w2[e0]
        we2[64:128, p, 14:24] = e_g[e1][:, None] * e_w2[e1]
    vals16["we2b"] = we2.reshape(128, 256)
    zw = np.zeros((128, 32), f)
    zw[0:64, 2] = 1.0 / 64
    zw[64:128, 3] = 1.0 / 64
    vals16["zwb"] = zw
    wsb = np.zeros((48, 2, 128), f)
    for e in range(E):
        b, j, q = e // 8, (e % 8) // 2, e % 2
        wsb[32 + e, b, 32 * j + 4 + 10 * q:32 * j + 14 + 10 * q] = 1.0
    vals16["wsbb"] = wsb.reshape(48, 256)
    ms = np.zeros((128, NCLS), f)
    for j in range(4):
        for q in range(2):
            for cc in range(NCLS):
                ms[32 * j + 4 + 10 * q + cc, cc] = 1.0
    vals16["msumb"] = ms
    gw2 = np.einsum("ed,edc->ec", e_g, e_w2)
    cst = np.einsum("ed,edc->ec", e_beta, e_w2) + e_b2
    gw2c = np.zeros((2 * E, NCLS), f)
    gw2c[0:E] = -gw2
    gw2c[E:2 * E] = cst
    vals16["gw2cb"] = gw2c
    # stat gather: row 32j+2s+q -> col 4j+2s+q
    pg = np.zeros((128, 32), f)
    for j in range(4):
        for s in range(2):
            for q in range(2):
                pg[32 * j + 2 * s + q, 4 * j + 2 * s + q] = 1.0
    vals16["permg"] = pg

    w1 = np.asarray(inp["bb_w1"], np.float64)
    w1h = w1.astype(np.float16)
    w1l = (w1 - w1h.astype(np.float64)).astype(np.float16)
    ch16 = np.zeros((128, 388), np.float16)
    ch16[0:IN_F, 0:EMB] = w1h
    ch16[0:IN_F, EMB:2 * EMB] = w1l
    u0 = _wgU0_f64
    u0h = u0.astype(np.float16)
    u0l = (u0 - u0h.astype(np.float64)).astype(np.float16)
    ch16[0:EMB, 128:256] = u0h
    ch16[0:EMB, 256:384] = u0l
    ch16[0:64, 384] = np.float16(1.0 / 64)
    ch16[64:128, 385] = np.float16(1.0 / 64)

    cf32 = np.zeros((128, CF32_COLS), f)
    for name, (p, o, w) in CF32_OFF.items():
        cf32[0:p, o:o + w] = vals32[name]
    cb16 = np.zeros((128, CB16_COLS), bf)
    for name, (p, o, w) in CB16_OFF.items():
        cb16[0:p, o:o + w] = np.asarray(vals16[name], np.float64).astype(bf)
    return {"cf32": cf32, "cb16": cb16, "ch16": ch16}


def prep_user_tables(inp):
    """uV gather table [NUSERS,128] plus per-user gate tables B, D [NUSERS,E].
    All in float64 then rounded once to f32."""
    gU = np.asarray(inp["gU"], np.float64)
    gV = np.asarray(inp["gV"], np.float64)
    gb = np.asarray(inp["gb"], np.float64)
    ut = np.asarray(inp["ut"], np.float64)
    bb_g = np.asarray(inp["bb_g"], np.float64)
    bb_beta = np.asarray(inp["bb_beta"], np.float64)
    wgU = np.zeros((EMB, 128), np.float64)
    for e in range(E):
        wgU[:, e * RANK:(e + 1) * RANK] = gU[e]
    uV = np.einsum("ud,edr->uer", ut, gV).reshape(NUSERS, 128)  # [u, e*8+r]
    cg = (bb_g @ wgU).reshape(E, RANK)       # wgU^T g
    cb = (bb_beta @ wgU).reshape(E, RANK)    # wgU^T beta
    uV3 = uV.reshape(NUSERS, E, RANK)
    Btab = np.einsum("er,uer->ue", cg, uV3)
    Dtab = np.einsum("er,uer->ue", cb, uV3) + gb[None, :]
    return uV.astype(np.float32), Btab.astype(np.float32), Dtab.astype(np.float32)


def shard_inputs(x, user_ids, inp, b_core):
    """x [B,80] -> per-core [nt,80,512] feature-major; uV gathered+transposed;
    B/D tables gathered batch-major."""
    ncores = x.shape[0] // b_core
    nt = b_core // TN
    xr = x.astype(np.float64)
    xh = xr.astype(np.float16)
    xl = (xr - xh.astype(np.float64)).astype(np.float16)
    xhs = xh.reshape(ncores, nt, TN, IN_F).transpose(0, 1, 3, 2)
    xls = xl.reshape(ncores, nt, TN, IN_F).transpose(0, 1, 3, 2)
    xs = np.ascontiguousarray(np.concatenate([xhs, xls], axis=3))  # [.., 80, 1024]
    uV, Btab, Dtab = prep_user_tables(inp)
    u = uV[user_ids]                                   # [B, 128]
    us = np.ascontiguousarray(
        u.reshape(ncores, nt, TN, 128).transpose(0, 1, 3, 2))
    # batch-major: sample s at (row=s%128, ch=s//128); B and D side by side
    BD = np.concatenate([Btab[user_ids], Dtab[user_ids]], axis=-1)  # [B, 2E]
    BDg = BD.reshape(ncores, nt, NCH, 128, 2 * E)
    BDt = np.ascontiguousarray(BDg.transpose(0, 1, 3, 2, 4))  # [.., 128, NCH, 2E]
    return xs, us, BDt


_CACHE = {}


def _get_program(b_core, mmdt="hybrid"):
    key = (b_core, mmdt)
    if key not in _CACHE:
        _CACHE[key] = build_program(b_core, mmdt)
    return _CACHE[key]


def build_in_maps(inputs):
    x = np.asarray(inputs["x"], np.float64).reshape(B, IN_F)
    uids = np.asarray(inputs["user_ids"]).astype(np.int64)
    cns = prep_consts({k: np.asarray(v) for k, v in inputs.items()})
    xs, us, BDt = shard_inputs(x, uids, inputs, B_CORE)
    in_maps = []
    for k in range(NCORES):
        m = dict(cns)
        m["x"] = xs[k]
        m["u"] = us[k]
        m["BD"] = BDt[k]
        in_maps.append(m)
    return in_maps


def kernel(**inputs):
    from concourse.bass_utils import run_bass_kernel_spmd
    nc = _get_program(B_CORE)
    in_maps = build_in_maps(inputs)
    res = run_bass_kernel_spmd(nc, in_maps, core_ids=list(range(NCORES)))
    out = np.concatenate([r["out"].reshape(B_CORE, NCLS) for r in res.results], axis=0)
    return out.astype(np.float32)



# revision 8
# speedup vs baseline: 1.1299x; 1.1299x over previous
"""Trainium2 Bass kernel for nn_MoEClassifier (moe_routing) — batch-major rework.

Model (per sample):
  x[16,5] -> flat 80 -> fc1(80->64) gelu -> fc2(64->64) gelu -> LN -> h
  u = user_table[user_id]  (16)
  gate: g_e = sum_r (h @ gU[e])_r * (u @ gV[e])_r + gb_e ; top-2 softmax -> w
  experts (dense): z_e = gelu(h @ e_w1[e] + e_b1[e]); LN(z); lpe = z @ e_w2[e] + e_b2
  logits = sum_e w_e * lpe_e   (10 classes)

Key idea vs the previous version: the cost model charges a matmul only for its
MOVING operand columns.  Wherever a [small x 512]-sample matmul only extracts a
few per-sample scalars, we swap roles: the per-sample activations become the
stationary lhsT (one 128-sample block at a time) and the small weight matrix
moves.  Output lands batch-major (samples on partitions), where per-sample
scalars are per-partition scalars.

  - bb-LN stats:   4 swap-matmuls of 2 f32 cols   (was 2x512 fp16 cols)
  - gate seg-sum:  4 swap-matmuls of 16 f32 cols  (was 512 f32 cols)
  - expert fc2:    64 swap-matmuls of 22+2 bf16 cols (was 16x512), with the
    LN mu-term folded into the weights: we2' = g*w2 - (g@w2)/64, so
    lpe = rs*(z @ we2') + cst.  mu/m2 ride as extra columns (z^2 lhsT for m2).
  - combine:       ws/w transposed once [128,32]->[32,128], then one swap-matmul
    against a 0/1+cst expansion produces per-sample per-column weights AND the
    sum_e w_e*cst_e term; elementwise multiply + log-tree reduce finishes.

Precision: identical-or-better vs previous version (fc2/psU0 back to plain f32
matmuls; stats from f32 h2 directly).  Gate top-2 verified 0 flips on this
input set in f64 emulation (validate_algebra.py).

PSUM (8 banks): bb-tag 2 (ps1/ps2/psU0/stp rotate), sm-tag 2 (psA/psBA/A_bm/
W32T/WSB rotate), z-tag 2 ([128,512] x8 rotate), P-tag 2 (two 1-bank tiles of
2 sample-blocks each, long-lived P8->P11).
"""
import sys, os

for _p in ("/opt/trn_rl_repo",):
    if _p not in sys.path:
        sys.path.insert(0, _p)

import numpy as np
from contextlib import ExitStack

import concourse.bass as bass
import concourse.tile as tile
from concourse import bacc, mybir

F32 = mybir.dt.float32
BF16 = mybir.dt.bfloat16
FP16 = mybir.dt.float16
I32 = mybir.dt.int32
AF = mybir.ActivationFunctionType
ALU = mybir.AluOpType

B = 131072
NCORES = 8
B_CORE = B // NCORES
IN_F = 80
EMB = 64
UDIM = 16
E = 16
RANK = 8
NCLS = 10
NUSERS = 1000
EPS_LN = 1e-5
TN = 512
NCH = TN // 128      # 4 sample-blocks of 128 per tile


def _bc(ap, n):
    """broadcast the (size-1) innermost dim of an AP to n via stride 0"""
    return ap.to_broadcast(list(ap.shape[:-1]) + [n])


# packed constant layouts: name -> (partitions, col offset, col width)
CF32_OFF = {
    "wbb2": (EMB, 0, EMB), "b1": (EMB, 64, 1), "b2": (EMB, 65, 1),
    "stat2": (128, 66, 2), "wgU0": (EMB, 68, 128), "gsum": (128, 196, E),
}
CF32_COLS = 212
CB16_OFF = {
    "identb": (128, 0, 128), "stlb": (3, 128, 128), "we1b": (65, 256, 1024),
    "we2s": (128, 1280, 176), "zw2": (128, 1456, 2), "xexp": (32, 1458, 202),
}
CB16_COLS = 1660


def build_program(b_core=B_CORE, mmdt="hybrid", bufs=None):
    ntiles = b_core // TN
    nc = bacc.Bacc("TRN2", target_bir_lowering=False, debug=False,
                   num_devices=NCORES)

    # ---------------- DRAM I/O ----------------
    d_x = nc.dram_tensor("x", [ntiles, IN_F, 2 * TN], FP16, kind="ExternalInput")
    d_u = nc.dram_tensor("u", [ntiles, 128, TN], F32, kind="ExternalInput")
    d_BD = nc.dram_tensor("BD", [ntiles, 128, NCH, 2 * E], F32, kind="ExternalInput")
    d_out = nc.dram_tensor("out", [ntiles, 128, NCH, NCLS], F32, kind="ExternalOutput")

    d_cf32 = nc.dram_tensor("cf32", [128, CF32_COLS], F32, kind="ExternalInput")
    d_cb16 = nc.dram_tensor("cb16", [128, CB16_COLS], BF16, kind="ExternalInput")
    d_ch16 = nc.dram_tensor("ch16", [128, 2 * EMB], FP16, kind="ExternalInput")

    # engine knobs: which engine runs the movable elementwise stages
    # (v=vector, p=pool, a=act where applicable)
    kn = {"z2pool": 5, "stf": "a", "statb": "v", "h2sq": "v", "gcp": "p",
          "passb": "p", "isw": "p", "w32": "p", "tree": "v"}
    for k in list(kn):
        v = os.environ.get("KN_" + k)
        if v is not None:
            kn[k] = int(v) if v.isdigit() else v

    bu = {"inp": 3, "work": 3, "scal": 4, "zsb": 18, "z2sb": 18, "osb": 3,
          "psbb": 2, "pssm": 2, "psz": 2, "psP": 2}
    for k in list(bu):
        v = os.environ.get("KB_" + k)
        if v:
            bu[k] = int(v)
    if bufs:
        bu.update(bufs)

    def eng(sel):
        return {"v": nc.vector, "p": nc.gpsimd}[sel]

    with tile.TileContext(nc) as tc, ExitStack() as ctx:
        cpool = ctx.enter_context(tc.tile_pool(name="consts", bufs=1))
        p_in = ctx.enter_context(tc.tile_pool(name="inp", bufs=bu["inp"]))
        p_w = ctx.enter_context(tc.tile_pool(name="work", bufs=bu["work"]))
        p_sc = ctx.enter_context(tc.tile_pool(name="scal", bufs=bu["scal"]))
        p_z = ctx.enter_context(tc.tile_pool(name="zsb", bufs=bu["zsb"]))
        p_z2 = ctx.enter_context(tc.tile_pool(name="z2sb", bufs=bu["z2sb"]))
        p_out = ctx.enter_context(tc.tile_pool(name="osb", bufs=bu["osb"]))
        ps_bb = ctx.enter_context(tc.tile_pool(name="psbb", bufs=bu["psbb"], space="PSUM"))
        ps_sm = ctx.enter_context(tc.tile_pool(name="pssm", bufs=bu["pssm"], space="PSUM"))
        ps_z = ctx.enter_context(tc.tile_pool(name="psz", bufs=bu["psz"], space="PSUM"))
        ps_P = ctx.enter_context(tc.tile_pool(name="psP", bufs=bu["psP"], space="PSUM"))

        # ------------- constants: packed DMAs, sliced views -------------
        t32 = cpool.tile([128, CF32_COLS], F32, tag="cf32", name="c_f32")
        nc.sync.dma_start(t32[:], d_cf32.ap())
        t16 = cpool.tile([128, CB16_COLS], BF16, tag="cb16", name="c_b16")
        nc.sync.dma_start(t16[:], d_cb16.ap())
        c = {}
        for name, (p, o, w) in CF32_OFF.items():
            c[name] = t32[0:p, o:o + w]
        for name, (p, o, w) in CB16_OFF.items():
            c[name] = t16[0:p, o:o + w]
        t_h16 = cpool.tile([128, 2 * EMB], FP16, tag="ch16", name="c_h16")
        nc.sync.dma_start(t_h16[:], d_ch16.ap())
        c["wbb1h"] = t_h16[0:IN_F, 0:EMB]
        c["wbb1l"] = t_h16[0:IN_F, EMB:2 * EMB]
        c["we1b"] = c["we1b"].rearrange("p (a b) -> p a b", a=8, b=128)
        c["we2s"] = c["we2s"].rearrange("p (a b) -> p a b", a=8, b=22)
        identb = c["identb"]

        def tile_body(it):
            # ==== P0: input DMAs + backbone fc1 + gelu ====
            x_fm = p_in.tile([IN_F, 2 * TN], FP16, tag="x_fm", bufs=3, name=f"x_{it}")
            nc.sync.dma_start(x_fm[:], d_x.ap()[it])
            u_fm = p_in.tile([128, TN], F32, tag="u_fm", bufs=6, name=f"u_{it}")
            nc.sync.dma_start(u_fm[:], d_u.ap()[it])
            BD_t = p_in.tile([128, NCH, 2 * E], F32, tag="BD", bufs=7, name=f"BD_{it}")
            nc.sync.dma_start(BD_t[:], d_BD.ap()[it])
            BT_t = BD_t[:, :, 0:E]
            DT_t = BD_t[:, :, E:2 * E]

            ps1 = ps_bb.tile([EMB, TN], F32, tag="bb", name=f"ps1_{it}")
            nc.tensor.matmul(ps1[:], c["wbb1h"], x_fm[:, 0:TN], start=True, stop=False)
            nc.tensor.matmul(ps1[:], c["wbb1h"], x_fm[:, TN:2 * TN], start=False, stop=False)
            nc.tensor.matmul(ps1[:], c["wbb1l"], x_fm[:, 0:TN], start=False, stop=True)
            h1 = p_w.tile([EMB, TN], F32, tag="h1", bufs=3, name=f"h1_{it}")
            nc.scalar.activation(h1[:], ps1[:], AF.Gelu, bias=c["b1"])

            yield  # ==== P1: backbone fc2 (f32) + gelu ====
            ps2 = ps_bb.tile([EMB, TN], F32, tag="bb", name=f"ps2_{it}")
            nc.tensor.matmul(ps2[:], c["wbb2"], h1[:], start=True, stop=True)
            h2s = p_w.tile([128, TN], F32, tag="h2s", bufs=5, name=f"h2s_{it}")
            nc.scalar.activation(h2s[0:EMB, :], ps2[:], AF.Gelu, bias=c["b2"])

            yield  # ==== P2: h2^2 + bb-LN stats (swap matmuls) ====
            eng(kn["h2sq"]).tensor_tensor(h2s[EMB:128, :], h2s[0:EMB, :],
                                          h2s[0:EMB, :], op=ALU.mult)
            psA = ps_sm.tile([128, NCH, 2], F32, tag="sm", name=f"psA_{it}")
            for b in range(NCH):
                nc.tensor.matmul(psA[:, b, :], h2s[:, 128 * b:128 * (b + 1)],
                                 c["stat2"], start=True, stop=True)
            sA = p_sc.tile([128, NCH, 2], F32, tag="sA", bufs=3, name=f"sA_{it}")
            nc.vector.tensor_copy(sA[:], psA[:])

            yield  # ==== P3: pass A (bb LN scalars, batch-major) ====
            tmpA = p_sc.tile([128, NCH], F32, tag="tmpA", bufs=3, name=f"tmpA_{it}")
            nc.vector.tensor_tensor(tmpA[:], sA[:, :, 0], sA[:, :, 0], op=ALU.mult)
            vA = p_sc.tile([128, NCH], F32, tag="vA", bufs=3, name=f"vA_{it}")
            nc.vector.scalar_tensor_tensor(vA[:], sA[:, :, 1], EPS_LN, tmpA[:],
                                           op0=ALU.add, op1=ALU.subtract)
            backA = p_sc.tile([128, NCH, 2], F32, tag="backA", bufs=4, name=f"backA_{it}")
            rsA = backA[:, :, 0]
            _newton_rsqrt(nc, p_sc, vA[:], rsA, [128, NCH], f"nA_{it}", niter=2)
            nc.vector.tensor_tensor(backA[:, :, 1], rsA, sA[:, :, 0], op=ALU.mult)
            backAb = p_sc.tile([128, NCH, 3], BF16, tag="backAb", bufs=3, name=f"backAb_{it}")
            nc.vector.tensor_copy(backAb[:, :, 0:2], backA[:])
            nc.gpsimd.memset(backAb[:, :, 2], 1.0)

            yield  # ==== P4: gate A matmul (f32) + gprod; rs/p transpose + stf ====
            psU0 = ps_bb.tile([128, TN], F32, tag="bb", name=f"psU0_{it}")
            nc.tensor.matmul(psU0[:], c["wgU0"], h2s[0:EMB, :], start=True, stop=True)
            gprod = p_w.tile([128, TN], F32, tag="gprod", bufs=4, name=f"gprod_{it}")
            nc.vector.tensor_tensor(gprod[:], psU0[:], u_fm[:], op=ALU.mult)

            psBA = ps_sm.tile([3, TN], BF16, tag="sm", name=f"psBA_{it}")
            for b in range(NCH):
                nc.tensor.transpose(psBA[:, 128 * b:128 * (b + 1)],
                                    backAb[:, b, :], identb)
            stf = p_sc.tile([3, TN], BF16, tag="stf", bufs=3, name=f"stf_{it}")
            if kn["stf"] == "a":
                nc.scalar.copy(stf[:], psBA[:])
            else:
                nc.vector.tensor_copy(stf[:], psBA[:])

            yield  # ==== P5: gate seg-sum (swap) + g1t; stp broadcast + t1h/hb ====
            A_bm = ps_sm.tile([128, NCH, E], F32, tag="sm", name=f"Abm_{it}")
            for b in range(NCH):
                nc.tensor.matmul(A_bm[:, b, :], gprod[:, 128 * b:128 * (b + 1)],
                                 c["gsum"], start=True, stop=True)
            g1t = p_sc.tile([128, NCH, E], F32, tag="g1t", bufs=3, name=f"g1t_{it}")
            nc.vector.tensor_tensor(g1t[:], A_bm[:], _bc(backA[:, :, 0:1], E),
                                    op=ALU.mult)

            stp = ps_bb.tile([128, TN], F32, tag="bb", name=f"stp_{it}")
            nc.tensor.matmul(stp[:], c["stlb"], stf[:], start=True, stop=True)
            hb = p_w.tile([EMB + 1, TN], BF16, tag="hb", bufs=4, name=f"hb_{it}")
            t1h = p_w.tile([EMB, TN], BF16, tag="t1h", bufs=3, name=f"t1h_{it}")
            nc.vector.tensor_tensor(t1h[:], h2s[0:EMB, :], stp[0:EMB, :], op=ALU.mult)
            nc.vector.tensor_tensor(hb[0:EMB, :], t1h[:], stp[EMB:128, :],
                                    op=ALU.subtract)
            nc.gpsimd.memset(hb[EMB:EMB + 1, :], 1.0)

            yield  # ==== P6: gate g = g1t - p*B + D ====
            g2t = p_sc.tile([128, NCH, E], F32, tag="g2t", bufs=3, name=f"g2t_{it}")
            eng(kn["gcp"]).tensor_tensor(g2t[:], BT_t, _bc(backA[:, :, 1:2], E),
                                         op=ALU.mult)
            g3t = p_sc.tile([128, NCH, E], F32, tag="g3t", bufs=3, name=f"g3t_{it}")
            eng(kn["gcp"]).tensor_tensor(g3t[:], g1t[:], g2t[:], op=ALU.subtract)
            gcp = p_sc.tile([128, NCH, E], F32, tag="gcp", bufs=5, name=f"gcp_{it}")
            eng(kn["gcp"]).tensor_tensor(gcp[:], g3t[:], DT_t, op=ALU.add)

            yield  # ==== P7: experts fc1 + gelu + z^2 ====
            z_sb = []
            for p in range(8):
                zq = ps_z.tile([128, TN], F32, tag="z", name=f"zq_{it}_{p}")
                nc.tensor.matmul(zq[:], c["we1b"][:, p, :], hb[:], start=True, stop=True)
                z = p_z.tile([128, TN], BF16, tag="z_sb", bufs=bu["zsb"], name=f"z_{it}_{p}")
                nc.scalar.activation(z[:], zq[:], AF.Gelu)
                z_sb.append(z)
            z2_sb = []
            for p in range(8):
                z2 = p_z2.tile([128, TN], BF16, tag="z2_sb", bufs=bu["z2sb"],
                               name=f"z2_{it}_{p}")
                e2 = nc.gpsimd if p < int(kn["z2pool"]) else nc.vector
                e2.tensor_tensor(z2[:], z_sb[p][:], z_sb[p][:], op=ALU.mult)
                z2_sb.append(z2)

            yield  # ==== P8: expert fc2 swap-matmuls -> P (batch-major) + extract ====
            # P layout per half: [128, 2 blocks, 256] ; pair p at cols 24p..24p+24
            # cols: 0:10 cls_e0' , 10:20 cls_e1', 20 mu_e0, 21 mu_e1, 22:24 m2
            # P psum is intra-phase scratch: cls + stats copied to SBUF here.
            statB = p_sc.tile([128, NCH, 8, 4], F32, tag="statB", bufs=3,
                              name=f"statB_{it}")
            Pcls = []
            for h in range(2):
                Pt = ps_P.tile([128, 2, 256], F32, tag="P", name=f"P{h}_{it}")
                for bb in range(2):
                    b = 2 * h + bb
                    for p in range(8):
                        zsl = z_sb[p][:, 128 * b:128 * (b + 1)]
                        z2sl = z2_sb[p][:, 128 * b:128 * (b + 1)]
                        nc.tensor.matmul(Pt[:, bb, 24 * p:24 * p + 22], zsl,
                                         c["we2s"][:, p, :], start=True, stop=True)
                        nc.tensor.matmul(Pt[:, bb, 24 * p + 22:24 * p + 24], z2sl,
                                         c["zw2"], start=True, stop=True)
                Pv = Pt[:, :, 0:192].rearrange("p b (e k) -> p b e k", e=8, k=24)
                if kn["statb"] == "a":
                    nc.scalar.copy(statB[:, 2 * h:2 * h + 2], Pv[:, :, :, 20:24])
                else:
                    nc.vector.tensor_copy(statB[:, 2 * h:2 * h + 2], Pv[:, :, :, 20:24])
                pc = p_w.tile([128, 2, 8, 20], BF16, tag="pcls", bufs=8,
                              name=f"pc_{it}_{h}")
                if kn.get("pcls", "a") == "a":
                    nc.scalar.copy(pc[:], Pv[:, :, :, 0:20])
                else:
                    nc.vector.tensor_copy(pc[:], Pv[:, :, :, 0:20])
                Pcls.append(pc)

            yield  # ==== P9: pass B (expert LN rs) + top-2 gate ====
            muB = statB[:, :, :, 0:2]
            m2B = statB[:, :, :, 2:4]
            tmpB = p_sc.tile([128, NCH, 8, 2], F32, tag="tmpB", bufs=3, name=f"tmpB_{it}")
            eng(kn["passb"]).tensor_tensor(tmpB[:], muB, muB, op=ALU.mult)
            vB = p_sc.tile([128, NCH, 8, 2], F32, tag="vB", bufs=3, name=f"vB_{it}")
            nc.vector.scalar_tensor_tensor(vB[:], m2B, EPS_LN, tmpB[:],
                                           op0=ALU.add, op1=ALU.subtract)
            rsB = p_sc.tile([128, NCH, 8, 2], F32, tag="rsB", bufs=3, name=f"rsB_{it}")
            _newton_rsqrt(nc, p_sc, vB[:], rsB[:], [128, NCH, 8, 2],
                          f"nB_{it}", niter=1)

            vm8 = p_sc.tile([128, NCH, 8], F32, tag="vm8", bufs=3, name=f"vm8_{it}")
            for ch in range(NCH):
                nc.vector.max(vm8[:, ch, :], gcp[:, ch, :])
            dg = p_sc.tile([128, NCH], F32, tag="dg", bufs=3, name=f"dg_{it}")
            nc.vector.tensor_tensor(dg[:], vm8[:, :, 0], vm8[:, :, 1], op=ALU.subtract)
            th = p_sc.tile([128, NCH], F32, tag="th", bufs=3, name=f"th_{it}")
            nc.scalar.activation(th[:], dg[:], AF.Tanh, scale=0.5)
            w12 = p_sc.tile([128, NCH, 2], F32, tag="w12", bufs=3, name=f"w12_{it}")
            nc.vector.tensor_scalar(w12[:, :, 0], th[:], 0.5, 0.5, op0=ALU.mult, op1=ALU.add)
            nc.vector.tensor_scalar(w12[:, :, 1], th[:], -0.5, 0.5, op0=ALU.mult, op1=ALU.add)

            is1 = p_sc.tile([128, NCH, E], F32, tag="is1", bufs=3, name=f"is1_{it}")
            nc.vector.tensor_tensor(is1[:], gcp[:], _bc(vm8[:, :, 0:1], E),
                                    op=ALU.is_equal)
            is2 = p_sc.tile([128, NCH, E], F32, tag="is2", bufs=3, name=f"is2_{it}")
            nc.vector.tensor_tensor(is2[:], gcp[:], _bc(vm8[:, :, 1:2], E),
                                    op=ALU.is_equal)
            w1t = p_sc.tile([128, NCH, E], F32, tag="w1t", bufs=3, name=f"w1t_{it}")
            eng(kn["isw"]).tensor_tensor(w1t[:], is1[:], _bc(w12[:, :, 0:1], E),
                                         op=ALU.mult)
            w2t = p_sc.tile([128, NCH, E], F32, tag="w2t", bufs=3, name=f"w2t_{it}")
            eng(kn["isw"]).tensor_tensor(w2t[:], is2[:], _bc(w12[:, :, 1:2], E),
                                         op=ALU.mult)

            yield  # ==== P10: W32 = [ws | w] build + transpose to feature-major ====
            W32 = p_sc.tile([128, NCH, 32], BF16, tag="W32", bufs=3, name=f"W32_{it}")
            wsum = p_sc.tile([128, NCH, E], F32, tag="wsum", bufs=3, name=f"wsum_{it}")
            eng(kn["w32"]).tensor_tensor(wsum[:], w1t[:], w2t[:], op=ALU.add)
            rsBf = rsB[:].rearrange("p c e q -> p c (e q)")
            eng(kn["w32"]).tensor_tensor(W32[:, :, 0:16], wsum[:], rsBf, op=ALU.mult)
            nc.vector.tensor_copy(W32[:, :, 16:32], wsum[:])

            psW = ps_sm.tile([32, TN], BF16, tag="sm", name=f"psW_{it}")
            for b in range(NCH):
                nc.tensor.transpose(psW[:, 128 * b:128 * (b + 1)], W32[:, b, :],
                                    identb)
            W32T = p_sc.tile([32, TN], BF16, tag="W32T", bufs=3, name=f"W32T_{it}")
            nc.vector.tensor_copy(W32T[:], psW[:])

            yield  # ==== P11: WSB swap-matmul + prod + cst extract ====
            prods = []
            for h in range(2):
                WSB = ps_sm.tile([128, 2, 202], F32, tag="sm", name=f"WSB_{it}_{h}")
                for bb in range(2):
                    b = 2 * h + bb
                    nc.tensor.matmul(WSB[:, bb, :], W32T[:, 128 * b:128 * (b + 1)],
                                     c["xexp"], start=True, stop=True)
                Wv = WSB[:, :, 0:192].rearrange("p b (e k) -> p b e k", e=8, k=24)
                pr = p_w.tile([128, 2, 8, 20], BF16, tag="pr", bufs=6,
                              name=f"pr_{it}_{h}")
                nc.vector.tensor_tensor(pr[:], Wv[:, :, :, 0:20], Pcls[h][:],
                                        op=ALU.mult)
                cstt = p_sc.tile([128, 2, NCLS], F32, tag="cstt", bufs=6,
                                 name=f"cstt_{it}_{h}")
                nc.vector.tensor_copy(cstt[:], WSB[:, :, 192:202])
                prods.append((pr, cstt))

            yield  # ==== P12: tree reduce + cst term + output ====
            osb = p_out.tile([128, NCH, NCLS], F32, tag="osb", bufs=3, name=f"osb_{it}")
            for h in range(2):
                pr, cstt = prods[h]
                te = eng(kn["tree"]) if kn["tree"] != "a" else nc.vector
                ta = p_sc.tile([128, 2, 4, 20], BF16, tag="ta", bufs=3,
                               name=f"ta_{it}_{h}")
                te.tensor_tensor(ta[:], pr[:, :, 0:4, :], pr[:, :, 4:8, :], op=ALU.add)
                tb = p_sc.tile([128, 2, 2, 20], BF16, tag="tb", bufs=3,
                               name=f"tb_{it}_{h}")
                te.tensor_tensor(tb[:], ta[:, :, 0:2, :], ta[:, :, 2:4, :], op=ALU.add)
                td = p_sc.tile([128, 2, 20], BF16, tag="td", bufs=3,
                               name=f"td_{it}_{h}")
                te.tensor_tensor(td[:], tb[:, :, 0, :], tb[:, :, 1, :], op=ALU.add)
                tf = p_sc.tile([128, 2, NCLS], BF16, tag="tf", bufs=3,
                               name=f"tf_{it}_{h}")
                te.tensor_tensor(tf[:], td[:, :, 0:10], td[:, :, 10:20], op=ALU.add)
                nc.vector.tensor_tensor(osb[:, 2 * h:2 * h + 2, :], tf[:],
                                        cstt[:], op=ALU.add)

            yield  # ==== P13: output DMA ====
            nc.sync.dma_start(d_out.ap()[it], osb[:])

        NPH = 14
        gens = {}
        for k in range(ntiles + NPH - 1):
            if k < ntiles:
                gens[k] = tile_body(k)
            for idx in sorted(gens):
                if next(gens[idx], StopIteration) is StopIteration:
                    del gens[idx]

    nc.compile()
    return nc


def _newton_rsqrt(nc, pool, v_ap, out_ap, shape, tag, niter=2, eng=None):
    """out = 1/sqrt(v) via quake seed + Newton iterations."""
    eng = eng or nc.vector
    r = pool.tile(shape, F32, tag=tag[:3] + "_r", name=tag + "_r")
    t = pool.tile(shape, F32, tag=tag[:3] + "_t", name=tag + "_t")
    eng.tensor_scalar(r[:].bitcast(I32), v_ap.bitcast(I32), 1, None,
                      op0=ALU.logical_shift_right)
    eng.tensor_scalar(r[:].bitcast(I32), r[:].bitcast(I32), -1, 0x5F3759DF,
                      op0=ALU.mult, op1=ALU.add)
    for i in range(niter):
        dst = out_ap if i == niter - 1 else r[:]
        eng.tensor_tensor(t[:], r[:], r[:], op=ALU.mult)
        eng.scalar_tensor_tensor(t[:], t[:], -0.5, v_ap, op0=ALU.mult, op1=ALU.mult)
        eng.scalar_tensor_tensor(dst, t[:], 1.5, r[:], op0=ALU.add, op1=ALU.mult)


# ---------------------------------------------------------------------------
# host-side weight prep
# ---------------------------------------------------------------------------
def prep_consts(inp):
    f = np.float32
    import ml_dtypes
    bf = ml_dtypes.bfloat16
    e_w1, e_b1 = np.asarray(inp["e_w1"], f), np.asarray(inp["e_b1"], f)
    e_g = np.asarray(inp["e_g"], np.float64)
    e_beta = np.asarray(inp["e_beta"], np.float64)
    e_w2, e_b2 = np.asarray(inp["e_w2"], np.float64), np.asarray(inp["e_b2"], np.float64)
    bb_g = np.asarray(inp["bb_g"], np.float64)
    bb_beta = np.asarray(inp["bb_beta"], np.float64)
    gU = np.asarray(inp["gU"], np.float64)

    vals32 = {}
    vals32["wbb2"] = np.asarray(inp["bb_w2"], f)
    vals32["b1"] = np.asarray(inp["bb_b1"], f).reshape(EMB, 1)
    vals32["b2"] = np.asarray(inp["bb_b2"], f).reshape(EMB, 1)
    st = np.zeros((128, 2), f)
    st[0:64, 0] = 1.0 / 64
    st[64:128, 1] = 1.0 / 64
    vals32["stat2"] = st
    wgU0 = np.zeros((EMB, 128), np.float64)
    for e in range(E):
        wgU0[:, e * RANK:(e + 1) * RANK] = gU[e] * bb_g[:, None]
    vals32["wgU0"] = wgU0.astype(f)
    gs = np.zeros((128, E), f)
    for e in range(E):
        gs[e * RANK:(e + 1) * RANK, e] = 1.0
    vals32["gsum"] = gs

    vals16 = {}
    vals16["identb"] = np.eye(128, dtype=f)
    # stp rows: [rs; p; 1] -> stp[0:64]=g*rs ; stp[64:128]=g*p - beta
    stl = np.zeros((3, 128), np.float64)
    stl[0, 0:64] = bb_g
    stl[1, 64:128] = bb_g
    stl[2, 64:128] = -bb_beta
    vals16["stlb"] = stl
    # we1 with bias row 64 (per pair: e0 cols 0:64, e1 cols 64:128)
    we1 = np.zeros((EMB + 1, 8, 128), f)
    for p in range(8):
        we1[0:EMB, p, 0:64] = e_w1[2 * p]
        we1[0:EMB, p, 64:128] = e_w1[2 * p + 1]
        we1[EMB, p, 0:64] = e_b1[2 * p]
        we1[EMB, p, 64:128] = e_b1[2 * p + 1]
    vals16["we1b"] = we1.reshape(EMB + 1, 1024)
    # fc2 swap weights: we2' = g*w2 - (g@w2)/64 ; mu cols 20,21
    gw2 = np.einsum("ed,edc->ec", e_g, e_w2)
    we2n = e_g[:, :, None] * e_w2 - gw2[:, None, :] / 64.0   # [E, 64, 10]
    we2 = np.zeros((128, 8, 22), np.float64)
    for p in range(8):
        e0, e1 = 2 * p, 2 * p + 1
        we2[0:64, p, 0:10] = we2n[e0]
        we2[64:128, p, 10:20] = we2n[e1]
        we2[0:64, p, 20] = 1.0 / 64
        we2[64:128, p, 21] = 1.0 / 64
    vals16["we2s"] = we2.reshape(128, 176)
    zw = np.zeros((128, 2), f)
    zw[0:64, 0] = 1.0 / 64
    zw[64:128, 1] = 1.0 / 64
    vals16["zw2"] = zw
    # xexp [32, 202]: rows 0:16 ws-expansion (0/1), rows 16:32 w->cst cols
    cst = np.einsum("ed,edc->ec", e_beta, e_w2) + e_b2
    xe = np.zeros((32, 202), np.float64)
    for e in range(E):
        p, q = e // 2, e % 2
        xe[e, 24 * p + 10 * q:24 * p + 10 * q + 10] = 1.0
        xe[16 + e, 192:202] = cst[e]
    vals16["xexp"] = xe

    w1 = np.asarray(inp["bb_w1"], np.float64)
    w1h = w1.astype(np.float16)
    w1l = (w1 - w1h.astype(np.float64)).astype(np.float16)
    ch16 = np.zeros((128, 2 * EMB), np.float16)
    ch16[0:IN_F, 0:EMB] = w1h
    ch16[0:IN_F, EMB:2 * EMB] = w1l

    cf32 = np.zeros((128, CF32_COLS), f)
    for name, (p, o, w) in CF32_OFF.items():
        cf32[0:p, o:o + w] = vals32[name]
    cb16 = np.zeros((128, CB16_COLS), bf)
    for name, (p, o, w) in CB16_OFF.items():
        cb16[0:p, o:o + w] = np.asarray(vals16[name], np.float64).astype(bf)
    return {"cf32": cf32, "cb16": cb16, "ch16": ch16}


def prep_user_tables(inp):
    """uV gather table [NUSERS,128] plus per-user gate tables B, D [NUSERS,E]."""
    gU = np.asarray(inp["gU"], np.float64)
    gV = np.asarray(inp["gV"], np.float64)
    gb = np.asarray(inp["gb"], np.float64)
    ut = np.asarray(inp["ut"], np.float64)
    bb_g = np.asarray(inp["bb_g"], np.float64)
    bb_beta = np.asarray(inp["bb_beta"], np.float64)
    wgU = np.zeros((EMB, 128), np.float64)
    for e in range(E):
        wgU[:, e * RANK:(e + 1) * RANK] = gU[e]
    uV = np.einsum("ud,edr->uer", ut, gV).reshape(NUSERS, 128)  # [u, e*8+r]
    cg = (bb_g @ wgU).reshape(E, RANK)       # wgU^T g
    cb = (bb_beta @ wgU).reshape(E, RANK)    # wgU^T beta
    uV3 = uV.reshape(NUSERS, E, RANK)
    Btab = np.einsum("er,uer->ue", cg, uV3)
    Dtab = np.einsum("er,uer->ue", cb, uV3) + gb[None, :]
    return uV.astype(np.float32), Btab.astype(np.float32), Dtab.astype(np.float32)


def shard_inputs(x, user_ids, inp, b_core):
    """x [B,80] -> per-core [nt,80,1024] fp16 hi|lo feature-major;
    uV gathered+transposed; B/D tables gathered batch-major."""
    ncores = x.shape[0] // b_core
    nt = b_core // TN
    xr = x.astype(np.float64)
    xh = xr.astype(np.float16)
    xl = (xr - xh.astype(np.float64)).astype(np.float16)
    xhs = xh.reshape(ncores, nt, TN, IN_F).transpose(0, 1, 3, 2)
    xls = xl.reshape(ncores, nt, TN, IN_F).transpose(0, 1, 3, 2)
    xs = np.ascontiguousarray(np.concatenate([xhs, xls], axis=3))  # [.., 80, 1024]
    uV, Btab, Dtab = prep_user_tables(inp)
    u = uV[user_ids]                                   # [B, 128]
    us = np.ascontiguousarray(
        u.reshape(ncores, nt, TN, 128).transpose(0, 1, 3, 2))
    # batch-major: sample s at (row=s%128, ch=s//128); B and D side by side
    BD = np.concatenate([Btab[user_ids], Dtab[user_ids]], axis=-1)  # [B, 2E]
    BDg = BD.reshape(ncores, nt, NCH, 128, 2 * E)
    BDt = np.ascontiguousarray(BDg.transpose(0, 1, 3, 2, 4))  # [.., 128, NCH, 2E]
    return xs, us, BDt


_CACHE = {}


def _get_program(b_core, mmdt="hybrid"):
    key = (b_core, mmdt)
    if key not in _CACHE:
        _CACHE[key] = build_program(b_core, mmdt)
    return _CACHE[key]


def build_in_maps(inputs):
    x = np.asarray(inputs["x"], np.float64).reshape(B, IN_F)
    uids = np.asarray(inputs["user_ids"]).astype(np.int64)
    cns = prep_consts({k: np.asarray(v) for k, v in inputs.items()})
    xs, us, BDt = shard_inputs(x, uids, inputs, B_CORE)
    in_maps = []
    for k in range(NCORES):
        m = dict(cns)
        m["x"] = xs[k]
        m["u"] = us[k]
        m["BD"] = BDt[k]
        in_maps.append(m)
    return in_maps


def kernel(**inputs):
    from concourse.bass_utils import run_bass_kernel_spmd
    nc = _get_program(B_CORE)
    in_maps = build_in_maps(inputs)
    res = run_bass_kernel_spmd(nc, in_maps, core_ids=list(range(NCORES)))
    nt = B_CORE // TN
    # out [nt, 128, NCH, NCLS]: sample = it*TN + ch*128 + row
    outs = []
    for r in res.results:
        o = r["out"].reshape(nt, 128, NCH, NCLS).transpose(0, 2, 1, 3)
        outs.append(o.reshape(B_CORE, NCLS))
    return np.concatenate(outs, axis=0).astype(np.float32)


# revision 14
# speedup vs baseline: 1.1628x; 1.0291x over previous
"""Trainium2 Bass kernel for nn_MoEClassifier (moe_routing) — batch-major rework.

Model (per sample):
  x[16,5] -> flat 80 -> fc1(80->64) gelu -> fc2(64->64) gelu -> LN -> h
  u = user_table[user_id]  (16)
  gate: g_e = sum_r (h @ gU[e])_r * (u @ gV[e])_r + gb_e ; top-2 softmax -> w
  experts (dense): z_e = gelu(h @ e_w1[e] + e_b1[e]); LN(z); lpe = z @ e_w2[e] + e_b2
  logits = sum_e w_e * lpe_e   (10 classes)

Key idea vs the previous version: the cost model charges a matmul only for its
MOVING operand columns.  Wherever a [small x 512]-sample matmul only extracts a
few per-sample scalars, we swap roles: the per-sample activations become the
stationary lhsT (one 128-sample block at a time) and the small weight matrix
moves.  Output lands batch-major (samples on partitions), where per-sample
scalars are per-partition scalars.

  - bb-LN stats:   4 swap-matmuls of 2 f32 cols   (was 2x512 fp16 cols)
  - gate seg-sum:  4 swap-matmuls of 16 f32 cols  (was 512 f32 cols)
  - expert fc2:    64 swap-matmuls of 22+2 bf16 cols (was 16x512), with the
    LN mu-term folded into the weights: we2' = g*w2 - (g@w2)/64, so
    lpe = rs*(z @ we2') + cst.  mu/m2 ride as extra columns (z^2 lhsT for m2).
  - combine:       ws/w transposed once [128,32]->[32,128], then one swap-matmul
    against a 0/1+cst expansion produces per-sample per-column weights AND the
    sum_e w_e*cst_e term; elementwise multiply + log-tree reduce finishes.

Precision: identical-or-better vs previous version (fc2/psU0 back to plain f32
matmuls; stats from f32 h2 directly).  Gate top-2 verified 0 flips on this
input set in f64 emulation (validate_algebra.py).

PSUM (8 banks): bb-tag 2 (ps1/ps2/psU0/stp rotate), sm-tag 2 (psA/psBA/A_bm/
W32T/WSB rotate), z-tag 2 ([128,512] x8 rotate), P-tag 2 (two 1-bank tiles of
2 sample-blocks each, long-lived P8->P11).
"""
import sys, os

for _p in ("/opt/trn_rl_repo",):
    if _p not in sys.path:
        sys.path.insert(0, _p)

import numpy as np
from contextlib import ExitStack

import concourse.bass as bass
import concourse.tile as tile
from concourse import bacc, mybir

F32 = mybir.dt.float32
BF16 = mybir.dt.bfloat16
FP16 = mybir.dt.float16
I32 = mybir.dt.int32
AF = mybir.ActivationFunctionType
ALU = mybir.AluOpType

B = 131072
NCORES = 8
B_CORE = B // NCORES
IN_F = 80
EMB = 64
UDIM = 16
E = 16
RANK = 8
NCLS = 10
NUSERS = 1000
EPS_LN = 1e-5
TN = 512
NCH = TN // 128      # 4 sample-blocks of 128 per tile


def _bc(ap, n):
    """broadcast the (size-1) innermost dim of an AP to n via stride 0"""
    return ap.to_broadcast(list(ap.shape[:-1]) + [n])


# packed constant layouts: name -> (partitions, col offset, col width)
CF32_OFF = {
    "wbb2": (EMB, 0, EMB), "b1": (EMB, 64, 1), "b2": (EMB, 65, 1),
    "stat2": (128, 66, 2), "wgU0": (EMB, 68, 128), "gsum": (128, 196, E),
}
CF32_COLS = 212
CB16_OFF = {
    "identb": (128, 0, 128), "stlb": (3, 128, 128), "we1b": (65, 256, 1024),
    "we2s": (128, 1280, 176), "zw2": (128, 1456, 2), "cstb": (E, 1458, NCLS),
}
CB16_COLS = 1468


def build_program(b_core=B_CORE, mmdt="hybrid", bufs=None):
    ntiles = b_core // TN
    nc = bacc.Bacc("TRN2", target_bir_lowering=False, debug=False,
                   num_devices=NCORES)

    # ---------------- DRAM I/O ----------------
    d_x = nc.dram_tensor("x", [ntiles, IN_F, 2 * TN], FP16, kind="ExternalInput")
    d_u = nc.dram_tensor("u", [ntiles, 128, TN], F32, kind="ExternalInput")
    d_BD = nc.dram_tensor("BD", [ntiles, 128, NCH, 2 * E], F32, kind="ExternalInput")
    d_out = nc.dram_tensor("out", [ntiles, 128, NCH, NCLS], F32, kind="ExternalOutput")

    d_cf32 = nc.dram_tensor("cf32", [128, CF32_COLS], F32, kind="ExternalInput")
    d_cb16 = nc.dram_tensor("cb16", [128, CB16_COLS], BF16, kind="ExternalInput")
    d_ch16 = nc.dram_tensor("ch16", [128, 2 * EMB], FP16, kind="ExternalInput")

    # engine knobs: which engine runs the movable elementwise stages
    # (v=vector, p=pool, a=act where applicable)
    kn = {"z2pool": 1, "stf": "a", "statb": "a", "sa": "a", "h2sq": "v",
          "gcp": "p", "passb": "p", "isw": "v", "w32": "v", "tree": "p",
          "cstt": "v", "wt": "v"}
    for k in list(kn):
        v = os.environ.get("KN_" + k)
        if v is not None:
            kn[k] = int(v) if v.isdigit() else v

    bu = {"inp": 3, "work": 3, "scal": 4, "zsb": 18, "z2sb": 18, "osb": 3,
          "psbb": 2, "pssm": 2, "psz": 2, "psP": 2}
    for k in list(bu):
        v = os.environ.get("KB_" + k)
        if v:
            bu[k] = int(v)
    if bufs:
        bu.update(bufs)

    def eng(sel):
        return {"v": nc.vector, "p": nc.gpsimd}[sel]

    with tile.TileContext(nc) as tc, ExitStack() as ctx:
        cpool = ctx.enter_context(tc.tile_pool(name="consts", bufs=1))
        p_in = ctx.enter_context(tc.tile_pool(name="inp", bufs=bu["inp"]))
        p_w = ctx.enter_context(tc.tile_pool(name="work", bufs=bu["work"]))
        p_sc = ctx.enter_context(tc.tile_pool(name="scal", bufs=bu["scal"]))
        p_z = ctx.enter_context(tc.tile_pool(name="zsb", bufs=bu["zsb"]))
        p_z2 = ctx.enter_context(tc.tile_pool(name="z2sb", bufs=bu["z2sb"]))
        p_out = ctx.enter_context(tc.tile_pool(name="osb", bufs=bu["osb"]))
        ps_bb = ctx.enter_context(tc.tile_pool(name="psbb", bufs=bu["psbb"], space="PSUM"))
        ps_sm = ctx.enter_context(tc.tile_pool(name="pssm", bufs=bu["pssm"], space="PSUM"))
        ps_z = ctx.enter_context(tc.tile_pool(name="psz", bufs=bu["psz"], space="PSUM"))
        ps_P = ctx.enter_context(tc.tile_pool(name="psP", bufs=bu["psP"], space="PSUM"))

        # ------------- constants: packed DMAs, sliced views -------------
        t32 = cpool.tile([128, CF32_COLS], F32, tag="cf32", name="c_f32")
        nc.sync.dma_start(t32[:], d_cf32.ap())
        t16 = cpool.tile([128, CB16_COLS], BF16, tag="cb16", name="c_b16")
        nc.sync.dma_start(t16[:], d_cb16.ap())
        c = {}
        for name, (p, o, w) in CF32_OFF.items():
            c[name] = t32[0:p, o:o + w]
        for name, (p, o, w) in CB16_OFF.items():
            c[name] = t16[0:p, o:o + w]
        t_h16 = cpool.tile([128, 2 * EMB], FP16, tag="ch16", name="c_h16")
        nc.sync.dma_start(t_h16[:], d_ch16.ap())
        c["wbb1h"] = t_h16[0:IN_F, 0:EMB]
        c["wbb1l"] = t_h16[0:IN_F, EMB:2 * EMB]
        c["we1b"] = c["we1b"].rearrange("p (a b) -> p a b", a=8, b=128)
        c["we2s"] = c["we2s"].rearrange("p (a b) -> p a b", a=8, b=22)
        identb = c["identb"]

        def tile_body(it):
            # ==== P0: input DMAs + backbone fc1 + gelu ====
            x_fm = p_in.tile([IN_F, 2 * TN], FP16, tag="x_fm", bufs=3, name=f"x_{it}")
            nc.sync.dma_start(x_fm[:], d_x.ap()[it])
            u_fm = p_in.tile([128, TN], F32, tag="u_fm", bufs=6, name=f"u_{it}")
            nc.sync.dma_start(u_fm[:], d_u.ap()[it])
            BD_t = p_in.tile([128, NCH, 2 * E], F32, tag="BD", bufs=7, name=f"BD_{it}")
            nc.sync.dma_start(BD_t[:], d_BD.ap()[it])
            BT_t = BD_t[:, :, 0:E]
            DT_t = BD_t[:, :, E:2 * E]

            ps1 = ps_bb.tile([EMB, TN], F32, tag="bb", name=f"ps1_{it}")
            nc.tensor.matmul(ps1[:], c["wbb1h"], x_fm[:, 0:TN], start=True, stop=False)
            nc.tensor.matmul(ps1[:], c["wbb1h"], x_fm[:, TN:2 * TN], start=False, stop=False)
            nc.tensor.matmul(ps1[:], c["wbb1l"], x_fm[:, 0:TN], start=False, stop=True)
            h1 = p_w.tile([EMB, TN], F32, tag="h1", bufs=3, name=f"h1_{it}")
            nc.scalar.activation(h1[:], ps1[:], AF.Gelu, bias=c["b1"])

            yield  # ==== P1: backbone fc2 (f32) + gelu ====
            ps2 = ps_bb.tile([EMB, TN], F32, tag="bb", name=f"ps2_{it}")
            nc.tensor.matmul(ps2[:], c["wbb2"], h1[:], start=True, stop=True)
            h2s = p_w.tile([128, TN], F32, tag="h2s", bufs=5, name=f"h2s_{it}")
            nc.scalar.activation(h2s[0:EMB, :], ps2[:], AF.Gelu, bias=c["b2"])

            yield  # ==== P2: h2^2 + bb-LN stats (swap matmuls) ====
            eng(kn["h2sq"]).tensor_tensor(h2s[EMB:128, :], h2s[0:EMB, :],
                                          h2s[0:EMB, :], op=ALU.mult)
            psA = ps_sm.tile([128, NCH, 2], F32, tag="sm", name=f"psA_{it}")
            for b in range(NCH):
                nc.tensor.matmul(psA[:, b, :], h2s[:, 128 * b:128 * (b + 1)],
                                 c["stat2"], start=True, stop=True)
            sA = p_sc.tile([128, NCH, 2], F32, tag="sA", bufs=3, name=f"sA_{it}")
            if kn.get("sa", "v") == "a":
                nc.scalar.copy(sA[:], psA[:])
            else:
                nc.vector.tensor_copy(sA[:], psA[:])

            yield  # ==== P3: pass A (bb LN scalars, batch-major) ====
            tmpA = p_sc.tile([128, NCH], F32, tag="tmpA", bufs=3, name=f"tmpA_{it}")
            nc.vector.tensor_tensor(tmpA[:], sA[:, :, 0], sA[:, :, 0], op=ALU.mult)
            vA = p_sc.tile([128, NCH], F32, tag="vA", bufs=3, name=f"vA_{it}")
            nc.vector.scalar_tensor_tensor(vA[:], sA[:, :, 1], EPS_LN, tmpA[:],
                                           op0=ALU.add, op1=ALU.subtract)
            backA = p_sc.tile([128, NCH, 2], F32, tag="backA", bufs=4, name=f"backA_{it}")
            rsA = backA[:, :, 0]
            _newton_rsqrt(nc, p_sc, vA[:], rsA, [128, NCH], f"nA_{it}", niter=2)
            nc.vector.tensor_tensor(backA[:, :, 1], rsA, sA[:, :, 0], op=ALU.mult)
            backAb = p_sc.tile([128, NCH, 3], BF16, tag="backAb", bufs=3, name=f"backAb_{it}")
            nc.vector.tensor_copy(backAb[:, :, 0:2], backA[:])
            nc.gpsimd.memset(backAb[:, :, 2], 1.0)

            yield  # ==== P4: gate A matmul (f32) + gprod; rs/p transpose + stf ====
            psU0 = ps_bb.tile([128, TN], F32, tag="bb", name=f"psU0_{it}")
            nc.tensor.matmul(psU0[:], c["wgU0"], h2s[0:EMB, :], start=True, stop=True)
            gprod = p_w.tile([128, TN], F32, tag="gprod", bufs=4, name=f"gprod_{it}")
            nc.vector.tensor_tensor(gprod[:], psU0[:], u_fm[:], op=ALU.mult)

            psBA = ps_sm.tile([3, TN], BF16, tag="sm", name=f"psBA_{it}")
            for b in range(NCH):
                nc.tensor.transpose(psBA[:, 128 * b:128 * (b + 1)],
                                    backAb[:, b, :], identb)
            stf = p_sc.tile([3, TN], BF16, tag="stf", bufs=3, name=f"stf_{it}")
            if kn["stf"] == "a":
                nc.scalar.copy(stf[:], psBA[:])
            else:
                nc.vector.tensor_copy(stf[:], psBA[:])

            yield  # ==== P5: gate seg-sum (swap) + g1t; stp broadcast + t1h/hb ====
            A_bm = ps_sm.tile([128, NCH, E], F32, tag="sm", name=f"Abm_{it}")
            for b in range(NCH):
                nc.tensor.matmul(A_bm[:, b, :], gprod[:, 128 * b:128 * (b + 1)],
                                 c["gsum"], start=True, stop=True)
            g1t = p_sc.tile([128, NCH, E], F32, tag="g1t", bufs=3, name=f"g1t_{it}")
            nc.vector.tensor_tensor(g1t[:], A_bm[:], _bc(backA[:, :, 0:1], E),
                                    op=ALU.mult)

            stp = ps_bb.tile([128, TN], F32, tag="bb", name=f"stp_{it}")
            nc.tensor.matmul(stp[:], c["stlb"], stf[:], start=True, stop=True)
            hb = p_w.tile([EMB + 1, TN], BF16, tag="hb", bufs=4, name=f"hb_{it}")
            t1h = p_w.tile([EMB, TN], BF16, tag="t1h", bufs=3, name=f"t1h_{it}")
            nc.vector.tensor_tensor(t1h[:], h2s[0:EMB, :], stp[0:EMB, :], op=ALU.mult)
            nc.vector.tensor_tensor(hb[0:EMB, :], t1h[:], stp[EMB:128, :],
                                    op=ALU.subtract)
            nc.gpsimd.memset(hb[EMB:EMB + 1, :], 1.0)

            yield  # ==== P6: gate g = g1t - p*B + D ====
            g2t = p_sc.tile([128, NCH, E], F32, tag="g2t", bufs=3, name=f"g2t_{it}")
            eng(kn["gcp"]).tensor_tensor(g2t[:], BT_t, _bc(backA[:, :, 1:2], E),
                                         op=ALU.mult)
            g3t = p_sc.tile([128, NCH, E], F32, tag="g3t", bufs=3, name=f"g3t_{it}")
            eng(kn["gcp"]).tensor_tensor(g3t[:], g1t[:], g2t[:], op=ALU.subtract)
            gcp = p_sc.tile([128, NCH, E], F32, tag="gcp", bufs=5, name=f"gcp_{it}")
            eng(kn["gcp"]).tensor_tensor(gcp[:], g3t[:], DT_t, op=ALU.add)

            yield  # ==== P7: experts fc1 + gelu + z^2 ; top-2 gate + cst term ====
            z_sb = []
            for p in range(8):
                zq = ps_z.tile([128, TN], F32, tag="z", name=f"zq_{it}_{p}")
                nc.tensor.matmul(zq[:], c["we1b"][:, p, :], hb[:], start=True, stop=True)
                z = p_z.tile([128, TN], BF16, tag="z_sb", bufs=bu["zsb"], name=f"z_{it}_{p}")
                nc.scalar.activation(z[:], zq[:], AF.Gelu)
                z_sb.append(z)
            z2_sb = []
            for p in range(8):
                z2 = p_z2.tile([128, TN], BF16, tag="z2_sb", bufs=bu["z2sb"],
                               name=f"z2_{it}_{p}")
                e2 = nc.gpsimd if p < int(kn["z2pool"]) else nc.vector
                e2.tensor_tensor(z2[:], z_sb[p][:], z_sb[p][:], op=ALU.mult)
                z2_sb.append(z2)

            # top-2 selection (from gcp, P6) and w weights
            vm8 = p_sc.tile([128, NCH, 8], F32, tag="vm8", bufs=3, name=f"vm8_{it}")
            for ch in range(NCH):
                nc.vector.max(vm8[:, ch, :], gcp[:, ch, :])
            dg = p_sc.tile([128, NCH], F32, tag="dg", bufs=3, name=f"dg_{it}")
            nc.vector.tensor_tensor(dg[:], vm8[:, :, 0], vm8[:, :, 1], op=ALU.subtract)
            th = p_sc.tile([128, NCH], F32, tag="th", bufs=3, name=f"th_{it}")
            nc.scalar.activation(th[:], dg[:], AF.Tanh, scale=0.5)
            w12 = p_sc.tile([128, NCH, 2], F32, tag="w12", bufs=3, name=f"w12_{it}")
            nc.vector.tensor_scalar(w12[:, :, 0], th[:], 0.5, 0.5, op0=ALU.mult, op1=ALU.add)
            nc.vector.tensor_scalar(w12[:, :, 1], th[:], -0.5, 0.5, op0=ALU.mult, op1=ALU.add)
            is1 = p_sc.tile([128, NCH, E], F32, tag="is1", bufs=3, name=f"is1_{it}")
            nc.vector.tensor_tensor(is1[:], gcp[:], _bc(vm8[:, :, 0:1], E),
                                    op=ALU.is_equal)
            is2 = p_sc.tile([128, NCH, E], F32, tag="is2", bufs=3, name=f"is2_{it}")
            nc.vector.tensor_tensor(is2[:], gcp[:], _bc(vm8[:, :, 1:2], E),
                                    op=ALU.is_equal)
            w1t = p_sc.tile([128, NCH, E], F32, tag="w1t", bufs=3, name=f"w1t_{it}")
            eng(kn["isw"]).tensor_tensor(w1t[:], is1[:], _bc(w12[:, :, 0:1], E),
                                         op=ALU.mult)
            w2t = p_sc.tile([128, NCH, E], F32, tag="w2t", bufs=3, name=f"w2t_{it}")
            eng(kn["isw"]).tensor_tensor(w2t[:], is2[:], _bc(w12[:, :, 1:2], E),
                                         op=ALU.mult)
            wsum = p_sc.tile([128, NCH, E], F32, tag="wsum", bufs=4, name=f"wsum_{it}")
            eng(kn["w32"]).tensor_tensor(wsum[:], w1t[:], w2t[:], op=ALU.add)
            # cst term: sum_e w_e * cst[e,c] via transpose + tiny swap-matmul
            wv16 = p_sc.tile([128, NCH, E], BF16, tag="wv16", bufs=3, name=f"wv16_{it}")
            nc.vector.tensor_copy(wv16[:], wsum[:])
            psWc = ps_sm.tile([E, TN], BF16, tag="sm", name=f"psWc_{it}")
            for b in range(NCH):
                nc.tensor.transpose(psWc[:, 128 * b:128 * (b + 1)], wv16[:, b, :],
                                    identb)
            wT = p_sc.tile([E, TN], BF16, tag="wT", bufs=3, name=f"wT_{it}")
            if kn.get("wt", "v") == "a":
                nc.scalar.copy(wT[:], psWc[:])
            else:
                nc.vector.tensor_copy(wT[:], psWc[:])

            yield  # ==== P8: fc2 swap-matmuls + stats + rs + weighted products ====
            cstD = ps_sm.tile([128, NCH, NCLS], F32, tag="sm", name=f"cstD_{it}")
            for b in range(NCH):
                nc.tensor.matmul(cstD[:, b, :], wT[:, 128 * b:128 * (b + 1)],
                                 c["cstb"], start=True, stop=True)
            cstt = p_sc.tile([128, NCH, NCLS], F32, tag="cstt", bufs=4,
                             name=f"cstt_{it}")
            if kn.get("cstt", "v") == "a":
                nc.scalar.copy(cstt[:], cstD[:])
            else:
                nc.vector.tensor_copy(cstt[:], cstD[:])

            # P layout per half: [128, 2 blocks, 256] ; pair p at cols 24p..24p+24
            # cols: 0:10 cls_e0', 10:20 cls_e1', 20 mu_e0, 21 mu_e1, 22:24 m2
            # P psum is intra-phase scratch.
            statB = p_sc.tile([128, NCH, 8, 4], F32, tag="statB", bufs=3,
                              name=f"statB_{it}")
            Pts = []
            for h in range(2):
                Pt = ps_P.tile([128, 2, 256], F32, tag="P", name=f"P{h}_{it}")
                for bb in range(2):
                    b = 2 * h + bb
                    for p in range(8):
                        zsl = z_sb[p][:, 128 * b:128 * (b + 1)]
                        z2sl = z2_sb[p][:, 128 * b:128 * (b + 1)]
                        nc.tensor.matmul(Pt[:, bb, 24 * p:24 * p + 22], zsl,
                                         c["we2s"][:, p, :], start=True, stop=True)
                        nc.tensor.matmul(Pt[:, bb, 24 * p + 22:24 * p + 24], z2sl,
                                         c["zw2"], start=True, stop=True)
                Pv = Pt[:, :, 0:192].rearrange("p b (e k) -> p b e k", e=8, k=24)
                if kn["statb"] == "a":
                    nc.scalar.copy(statB[:, 2 * h:2 * h + 2], Pv[:, :, :, 20:24])
                else:
                    nc.vector.tensor_copy(statB[:, 2 * h:2 * h + 2], Pv[:, :, :, 20:24])
                pc = p_w.tile([128, 2, 8, 2, 10], BF16, tag="pcls", bufs=8,
                              name=f"pc_{it}_{h}")
                Pcv = Pv[:, :, :, 0:20].rearrange("p b e (q c) -> p b e q c", q=2, c=10)
                if kn.get("pcls", "a") == "a":
                    nc.scalar.copy(pc[:], Pcv)
                else:
                    nc.vector.tensor_copy(pc[:], Pcv)
                Pts.append(pc)

            yield  # ==== P9: pass B (rs) + weighted products ====
            muB = statB[:, :, :, 0:2]
            m2B = statB[:, :, :, 2:4]
            tmpB = p_sc.tile([128, NCH, 8, 2], F32, tag="tmpB", bufs=3, name=f"tmpB_{it}")
            eng(kn["passb"]).tensor_tensor(tmpB[:], muB, muB, op=ALU.mult)
            vB = p_sc.tile([128, NCH, 8, 2], F32, tag="vB", bufs=3, name=f"vB_{it}")
            nc.vector.scalar_tensor_tensor(vB[:], m2B, EPS_LN, tmpB[:],
                                           op0=ALU.add, op1=ALU.subtract)
            rsB = p_sc.tile([128, NCH, 8, 2], F32, tag="rsB", bufs=3, name=f"rsB_{it}")
            _newton_rsqrt(nc, p_sc, vB[:], rsB[:], [128, NCH, 8, 2],
                          f"nB_{it}", niter=1)
            rsBf = rsB[:].rearrange("p c e q -> p c (e q)")
            wsb16 = p_sc.tile([128, NCH, E], BF16, tag="wsb16", bufs=3,
                              name=f"wsb16_{it}")
            nc.vector.tensor_tensor(wsb16[:], wsum[:], rsBf, op=ALU.mult)

            prods = []
            for h in range(2):
                wsv = wsb16[:, 2 * h:2 * h + 2, :].rearrange(
                    "p b (e q o) -> p b e q o", e=8, q=2, o=1)
                pr = p_w.tile([128, 2, 8, 2, 10], BF16, tag="pr", bufs=6,
                              name=f"pr_{it}_{h}")
                nc.vector.tensor_tensor(pr[:], Pts[h][:], _bc(wsv, 10), op=ALU.mult)
                prods.append(pr)

            yield  # ==== P10: tree reduce + cst add ====
            osb = p_out.tile([128, NCH, NCLS], F32, tag="osb", bufs=3, name=f"osb_{it}")
            for h in range(2):
                pr = prods[h]
                te = eng(kn["tree"]) if kn["tree"] != "a" else nc.vector
                ta = p_sc.tile([128, 2, 4, 2, 10], BF16, tag="ta", bufs=3,
                               name=f"ta_{it}_{h}")
                te.tensor_tensor(ta[:], pr[:, :, 0:4], pr[:, :, 4:8], op=ALU.add)
                tb = p_sc.tile([128, 2, 2, 2, 10], BF16, tag="tb", bufs=3,
                               name=f"tb_{it}_{h}")
                te.tensor_tensor(tb[:], ta[:, :, 0:2], ta[:, :, 2:4], op=ALU.add)
                td = p_sc.tile([128, 2, 2, 10], BF16, tag="td", bufs=3,
                               name=f"td_{it}_{h}")
                te.tensor_tensor(td[:], tb[:, :, 0], tb[:, :, 1], op=ALU.add)
                tf = p_sc.tile([128, 2, NCLS], BF16, tag="tf", bufs=3,
                               name=f"tf_{it}_{h}")
                te.tensor_tensor(tf[:], td[:, :, 0], td[:, :, 1], op=ALU.add)
                nc.vector.tensor_tensor(osb[:, 2 * h:2 * h + 2, :], tf[:],
                                        cstt[:, 2 * h:2 * h + 2, :], op=ALU.add)

            yield  # ==== P11: output DMA ====
            nc.sync.dma_start(d_out.ap()[it], osb[:])

        NPH = 12
        gens = {}
        for k in range(ntiles + NPH - 1):
            if k < ntiles:
                gens[k] = tile_body(k)
            for idx in sorted(gens):
                if next(gens[idx], StopIteration) is StopIteration:
                    del gens[idx]

    nc.compile()
    return nc


def _newton_rsqrt(nc, pool, v_ap, out_ap, shape, tag, niter=2, eng=None):
    """out = 1/sqrt(v) via quake seed + Newton iterations."""
    eng = eng or nc.vector
    r = pool.tile(shape, F32, tag=tag[:3] + "_r", name=tag + "_r")
    t = pool.tile(shape, F32, tag=tag[:3] + "_t", name=tag + "_t")
    eng.tensor_scalar(r[:].bitcast(I32), v_ap.bitcast(I32), 1, None,
                      op0=ALU.logical_shift_right)
    eng.tensor_scalar(r[:].bitcast(I32), r[:].bitcast(I32), -1, 0x5F3759DF,
                      op0=ALU.mult, op1=ALU.add)
    for i in range(niter):
        dst = out_ap if i == niter - 1 else r[:]
        eng.tensor_tensor(t[:], r[:], r[:], op=ALU.mult)
        eng.scalar_tensor_tensor(t[:], t[:], -0.5, v_ap, op0=ALU.mult, op1=ALU.mult)
        eng.scalar_tensor_tensor(dst, t[:], 1.5, r[:], op0=ALU.add, op1=ALU.mult)


# ---------------------------------------------------------------------------
# host-side weight prep
# ---------------------------------------------------------------------------
def prep_consts(inp):
    f = np.float32
    import ml_dtypes
    bf = ml_dtypes.bfloat16
    e_w1, e_b1 = np.asarray(inp["e_w1"], f), np.asarray(inp["e_b1"], f)
    e_g = np.asarray(inp["e_g"], np.float64)
    e_beta = np.asarray(inp["e_beta"], np.float64)
    e_w2, e_b2 = np.asarray(inp["e_w2"], np.float64), np.asarray(inp["e_b2"], np.float64)
    bb_g = np.asarray(inp["bb_g"], np.float64)
    bb_beta = np.asarray(inp["bb_beta"], np.float64)
    gU = np.asarray(inp["gU"], np.float64)

    vals32 = {}
    vals32["wbb2"] = np.asarray(inp["bb_w2"], f)
    vals32["b1"] = np.asarray(inp["bb_b1"], f).reshape(EMB, 1)
    vals32["b2"] = np.asarray(inp["bb_b2"], f).reshape(EMB, 1)
    st = np.zeros((128, 2), f)
    st[0:64, 0] = 1.0 / 64
    st[64:128, 1] = 1.0 / 64
    vals32["stat2"] = st
    wgU0 = np.zeros((EMB, 128), np.float64)
    for e in range(E):
        wgU0[:, e * RANK:(e + 1) * RANK] = gU[e] * bb_g[:, None]
    vals32["wgU0"] = wgU0.astype(f)
    gs = np.zeros((128, E), f)
    for e in range(E):
        gs[e * RANK:(e + 1) * RANK, e] = 1.0
    vals32["gsum"] = gs

    vals16 = {}
    vals16["identb"] = np.eye(128, dtype=f)
    # stp rows: [rs; p; 1] -> stp[0:64]=g*rs ; stp[64:128]=g*p - beta
    stl = np.zeros((3, 128), np.float64)
    stl[0, 0:64] = bb_g
    stl[1, 64:128] = bb_g
    stl[2, 64:128] = -bb_beta
    vals16["stlb"] = stl
    # we1 with bias row 64 (per pair: e0 cols 0:64, e1 cols 64:128)
    we1 = np.zeros((EMB + 1, 8, 128), f)
    for p in range(8):
        we1[0:EMB, p, 0:64] = e_w1[2 * p]
        we1[0:EMB, p, 64:128] = e_w1[2 * p + 1]
        we1[EMB, p, 0:64] = e_b1[2 * p]
        we1[EMB, p, 64:128] = e_b1[2 * p + 1]
    vals16["we1b"] = we1.reshape(EMB + 1, 1024)
    # fc2 swap weights: we2' = g*w2 - (g@w2)/64 ; mu cols 20,21
    gw2 = np.einsum("ed,edc->ec", e_g, e_w2)
    we2n = e_g[:, :, None] * e_w2 - gw2[:, None, :] / 64.0   # [E, 64, 10]
    we2 = np.zeros((128, 8, 22), np.float64)
    for p in range(8):
        e0, e1 = 2 * p, 2 * p + 1
        we2[0:64, p, 0:10] = we2n[e0]
        we2[64:128, p, 10:20] = we2n[e1]
        we2[0:64, p, 20] = 1.0 / 64
        we2[64:128, p, 21] = 1.0 / 64
    vals16["we2s"] = we2.reshape(128, 176)
    zw = np.zeros((128, 2), f)
    zw[0:64, 0] = 1.0 / 64
    zw[64:128, 1] = 1.0 / 64
    vals16["zw2"] = zw
    # xexp [32, 202]: rows 0:16 ws-expansion (0/1), rows 16:32 w->cst cols
    cst = np.einsum("ed,edc->ec", e_beta, e_w2) + e_b2
    vals16["cstb"] = cst

    w1 = np.asarray(inp["bb_w1"], np.float64)
    w1h = w1.astype(np.float16)
    w1l = (w1 - w1h.astype(np.float64)).astype(np.float16)
    ch16 = np.zeros((128, 2 * EMB), np.float16)
    ch16[0:IN_F, 0:EMB] = w1h
    ch16[0:IN_F, EMB:2 * EMB] = w1l

    cf32 = np.zeros((128, CF32_COLS), f)
    for name, (p, o, w) in CF32_OFF.items():
        cf32[0:p, o:o + w] = vals32[name]
    cb16 = np.zeros((128, CB16_COLS), bf)
    for name, (p, o, w) in CB16_OFF.items():
        cb16[0:p, o:o + w] = np.asarray(vals16[name], np.float64).astype(bf)
    return {"cf32": cf32, "cb16": cb16, "ch16": ch16}


def prep_user_tables(inp):
    """uV gather table [NUSERS,128] plus per-user gate tables B, D [NUSERS,E]."""
    gU = np.asarray(inp["gU"], np.float64)
    gV = np.asarray(inp["gV"], np.float64)
    gb = np.asarray(inp["gb"], np.float64)
    ut = np.asarray(inp["ut"], np.float64)
    bb_g = np.asarray(inp["bb_g"], np.float64)
    bb_beta = np.asarray(inp["bb_beta"], np.float64)
    wgU = np.zeros((EMB, 128), np.float64)
    for e in range(E):
        wgU[:, e * RANK:(e + 1) * RANK] = gU[e]
    uV = np.einsum("ud,edr->uer", ut, gV).reshape(NUSERS, 128)  # [u, e*8+r]
    cg = (bb_g @ wgU).reshape(E, RANK)       # wgU^T g
    cb = (bb_beta @ wgU).reshape(E, RANK)    # wgU^T beta
    uV3 = uV.reshape(NUSERS, E, RANK)
    Btab = np.einsum("er,uer->ue", cg, uV3)
    Dtab = np.einsum("er,uer->ue", cb, uV3) + gb[None, :]
    return uV.astype(np.float32), Btab.astype(np.float32), Dtab.astype(np.float32)


def shard_inputs(x, user_ids, inp, b_core):
    """x [B,80] -> per-core [nt,80,1024] fp16 hi|lo feature-major;
    uV gathered+transposed; B/D tables gathered batch-major."""
    ncores = x.shape[0] // b_core
    nt = b_core // TN
    xr = x.astype(np.float64)
    xh = xr.astype(np.float16)
    xl = (xr - xh.astype(np.float64)).astype(np.float16)
    xhs = xh.reshape(ncores, nt, TN, IN_F).transpose(0, 1, 3, 2)
    xls = xl.reshape(ncores, nt, TN, IN_F).transpose(0, 1, 3, 2)
    xs = np.ascontiguousarray(np.concatenate([xhs, xls], axis=3))  # [.., 80, 1024]
    uV, Btab, Dtab = prep_user_tables(inp)
    u = uV[user_ids]                                   # [B, 128]
    us = np.ascontiguousarray(
        u.reshape(ncores, nt, TN, 128).transpose(0, 1, 3, 2))
    # batch-major: sample s at (row=s%128, ch=s//128); B and D side by side
    BD = np.concatenate([Btab[user_ids], Dtab[user_ids]], axis=-1)  # [B, 2E]
    BDg = BD.reshape(ncores, nt, NCH, 128, 2 * E)
    BDt = np.ascontiguousarray(BDg.transpose(0, 1, 3, 2, 4))  # [.., 128, NCH, 2E]
    return xs, us, BDt


_CACHE = {}


def _get_program(b_core, mmdt="hybrid"):
    key = (b_core, mmdt)
    if key not in _CACHE:
        _CACHE[key] = build_program(b_core, mmdt)
    return _CACHE[key]


def build_in_maps(inputs):
    x = np.asarray(inputs["x"], np.float64).reshape(B, IN_F)
    uids = np.asarray(inputs["user_ids"]).astype(np.int64)
    cns = prep_consts({k: np.asarray(v) for k, v in inputs.items()})
    xs, us, BDt = shard_inputs(x, uids, inputs, B_CORE)
    in_maps = []
    for k in range(NCORES):
        m = dict(cns)
        m["x"] = xs[k]
        m["u"] = us[k]
        m["BD"] = BDt[k]
        in_maps.append(m)
    return in_maps


def kernel(**inputs):
    from concourse.bass_utils import run_bass_kernel_spmd
    nc = _get_program(B_CORE)
    in_maps = build_in_maps(inputs)
    res = run_bass_kernel_spmd(nc, in_maps, core_ids=list(range(NCORES)))
    nt = B_CORE // TN
    # out [nt, 128, NCH, NCLS]: sample = it*TN + ch*128 + row
    outs = []
    for r in res.results:
        o = r["out"].reshape(nt, 128, NCH, NCLS).transpose(0, 2, 1, 3)
        outs.append(o.reshape(B_CORE, NCLS))
    return np.concatenate(outs, axis=0).astype(np.float32)


# revision 19
# speedup vs baseline: 1.2330x; 1.0604x over previous
"""Trainium2 Bass kernel for nn_MoEClassifier (moe_routing) — batch-major rework.

Model (per sample):
  x[16,5] -> flat 80 -> fc1(80->64) gelu -> fc2(64->64) gelu -> LN -> h
  u = user_table[user_id]  (16)
  gate: g_e = sum_r (h @ gU[e])_r * (u @ gV[e])_r + gb_e ; top-2 softmax -> w
  experts (dense): z_e = gelu(h @ e_w1[e] + e_b1[e]); LN(z); lpe = z @ e_w2[e] + e_b2
  logits = sum_e w_e * lpe_e   (10 classes)

Key idea vs the previous version: the cost model charges a matmul only for its
MOVING operand columns.  Wherever a [small x 512]-sample matmul only extracts a
few per-sample scalars, we swap roles: the per-sample activations become the
stationary lhsT (one 128-sample block at a time) and the small weight matrix
moves.  Output lands batch-major (samples on partitions), where per-sample
scalars are per-partition scalars.

  - bb-LN stats:   4 swap-matmuls of 2 f32 cols   (was 2x512 fp16 cols)
  - gate seg-sum:  4 swap-matmuls of 16 f32 cols  (was 512 f32 cols)
  - expert fc2:    64 swap-matmuls of 22+2 bf16 cols (was 16x512), with the
    LN mu-term folded into the weights: we2' = g*w2 - (g@w2)/64, so
    lpe = rs*(z @ we2') + cst.  mu/m2 ride as extra columns (z^2 lhsT for m2).
  - combine:       ws/w transposed once [128,32]->[32,128], then one swap-matmul
    against a 0/1+cst expansion produces per-sample per-column weights AND the
    sum_e w_e*cst_e term; elementwise multiply + log-tree reduce finishes.

Precision: identical-or-better vs previous version (fc2/psU0 back to plain f32
matmuls; stats from f32 h2 directly).  Gate top-2 verified 0 flips on this
input set in f64 emulation (validate_algebra.py).

PSUM (8 banks): bb-tag 2 (ps1/ps2/psU0/stp rotate), sm-tag 2 (psA/psBA/A_bm/
W32T/WSB rotate), z-tag 2 ([128,512] x8 rotate), P-tag 2 (two 1-bank tiles of
2 sample-blocks each, long-lived P8->P11).
"""
import sys, os

for _p in ("/opt/trn_rl_repo",):
    if _p not in sys.path:
        sys.path.insert(0, _p)

import numpy as np
from contextlib import ExitStack

import concourse.bass as bass
import concourse.tile as tile
from concourse import bacc, mybir

F32 = mybir.dt.float32
BF16 = mybir.dt.bfloat16
FP16 = mybir.dt.float16
I32 = mybir.dt.int32
AF = mybir.ActivationFunctionType
ALU = mybir.AluOpType

B = 131072
NCORES = 8
B_CORE = B // NCORES
IN_F = 80
EMB = 64
UDIM = 16
E = 16
RANK = 8
NCLS = 10
NUSERS = 1000
EPS_LN = 1e-5
TN = 512
NCH = TN // 128      # 4 sample-blocks of 128 per tile


def _bc(ap, n):
    """broadcast the (size-1) innermost dim of an AP to n via stride 0"""
    return ap.to_broadcast(list(ap.shape[:-1]) + [n])


# packed constant layouts: name -> (partitions, col offset, col width)
CF32_OFF = {
    "wbb2": (EMB, 0, EMB), "b1": (EMB, 64, 1), "b2": (EMB, 65, 1),
    "stat2": (128, 66, 2), "wgU0": (EMB, 68, 128), "gsum": (128, 196, E),
}
CF32_COLS = 212
CB16_OFF = {
    "identb": (128, 0, 128), "stlb": (3, 128, 128), "we1b": (65, 256, 1024),
    "we2s": (128, 1280, 176), "zw2": (128, 1456, 2), "cstb": (E, 1458, NCLS),
}
CB16_COLS = 1468


def build_program(b_core=B_CORE, mmdt="hybrid", bufs=None):
    ntiles = b_core // TN
    nc = bacc.Bacc("TRN2", target_bir_lowering=False, debug=False,
                   num_devices=NCORES)

    # ---------------- DRAM I/O ----------------
    d_x = nc.dram_tensor("x", [ntiles, IN_F, 2 * TN], FP16, kind="ExternalInput")
    d_u = nc.dram_tensor("u", [ntiles, 128, TN], F32, kind="ExternalInput")
    d_BD = nc.dram_tensor("BD", [ntiles, 128, NCH, 2 * E], F32, kind="ExternalInput")
    d_out = nc.dram_tensor("out", [ntiles, 128, NCH, NCLS], F32, kind="ExternalOutput")

    d_cf32 = nc.dram_tensor("cf32", [128, CF32_COLS], F32, kind="ExternalInput")
    d_cb16 = nc.dram_tensor("cb16", [128, CB16_COLS], BF16, kind="ExternalInput")
    d_ch16 = nc.dram_tensor("ch16", [128, 2 * EMB], FP16, kind="ExternalInput")

    # engine knobs: which engine runs the movable elementwise stages
    # (v=vector, p=pool, a=act where applicable)
    kn = {"z2pool": 3, "stf": "a", "statb": "a", "sa": "a", "h2sq": "v",
          "gcp": "p", "passb": "p", "isw": "v", "w32": "v", "tree": "p",
          "cstt": "v", "wt": "v", "pcls": "a", "prps": 1, "z2act": 1,
          "rsa": "n", "rsb": "n"}
    for k in list(kn):
        v = os.environ.get("KN_" + k)
        if v is not None:
            kn[k] = int(v) if v.isdigit() else v

    bu = {"inp": 3, "work": 3, "scal": 4, "zsb": 18, "z2sb": 18, "osb": 3,
          "psbb": 2, "pssm": 2, "psz": 2, "psP": 2}
    for k in list(bu):
        v = os.environ.get("KB_" + k)
        if v:
            bu[k] = int(v)
    if bufs:
        bu.update(bufs)

    def eng(sel):
        return {"v": nc.vector, "p": nc.gpsimd}[sel]

    with tile.TileContext(nc) as tc, ExitStack() as ctx:
        cpool = ctx.enter_context(tc.tile_pool(name="consts", bufs=1))
        p_in = ctx.enter_context(tc.tile_pool(name="inp", bufs=bu["inp"]))
        p_w = ctx.enter_context(tc.tile_pool(name="work", bufs=bu["work"]))
        p_sc = ctx.enter_context(tc.tile_pool(name="scal", bufs=bu["scal"]))
        p_z = ctx.enter_context(tc.tile_pool(name="zsb", bufs=bu["zsb"]))
        p_z2 = ctx.enter_context(tc.tile_pool(name="z2sb", bufs=bu["z2sb"]))
        p_out = ctx.enter_context(tc.tile_pool(name="osb", bufs=bu["osb"]))
        ps_bb = ctx.enter_context(tc.tile_pool(name="psbb", bufs=bu["psbb"], space="PSUM"))
        ps_sm = ctx.enter_context(tc.tile_pool(name="pssm", bufs=bu["pssm"], space="PSUM"))
        ps_z = ctx.enter_context(tc.tile_pool(name="psz", bufs=bu["psz"], space="PSUM"))
        ps_P = ctx.enter_context(tc.tile_pool(name="psP", bufs=bu["psP"], space="PSUM"))

        # ------------- constants: packed DMAs, sliced views -------------
        t32 = cpool.tile([128, CF32_COLS], F32, tag="cf32", name="c_f32")
        nc.sync.dma_start(t32[:], d_cf32.ap())
        t16 = cpool.tile([128, CB16_COLS], BF16, tag="cb16", name="c_b16")
        nc.sync.dma_start(t16[:], d_cb16.ap())
        c = {}
        for name, (p, o, w) in CF32_OFF.items():
            c[name] = t32[0:p, o:o + w]
        for name, (p, o, w) in CB16_OFF.items():
            c[name] = t16[0:p, o:o + w]
        t_h16 = cpool.tile([128, 2 * EMB], FP16, tag="ch16", name="c_h16")
        nc.sync.dma_start(t_h16[:], d_ch16.ap())
        c["wbb1h"] = t_h16[0:IN_F, 0:EMB]
        c["wbb1l"] = t_h16[0:IN_F, EMB:2 * EMB]
        c["we1b"] = c["we1b"].rearrange("p (a b) -> p a b", a=8, b=128)
        c["we2s"] = c["we2s"].rearrange("p (a b) -> p a b", a=8, b=22)
        identb = c["identb"]

        def tile_body(it):
            # ==== P0: input DMAs + backbone fc1 + gelu ====
            x_fm = p_in.tile([IN_F, 2 * TN], FP16, tag="x_fm", bufs=3, name=f"x_{it}")
            nc.sync.dma_start(x_fm[:], d_x.ap()[it])
            u_fm = p_in.tile([128, TN], F32, tag="u_fm", bufs=6, name=f"u_{it}")
            nc.sync.dma_start(u_fm[:], d_u.ap()[it])
            BD_t = p_in.tile([128, NCH, 2 * E], F32, tag="BD", bufs=7, name=f"BD_{it}")
            nc.sync.dma_start(BD_t[:], d_BD.ap()[it])
            BT_t = BD_t[:, :, 0:E]
            DT_t = BD_t[:, :, E:2 * E]

            ps1 = ps_bb.tile([EMB, TN], F32, tag="bb", name=f"ps1_{it}")
            nc.tensor.matmul(ps1[:], c["wbb1h"], x_fm[:, 0:TN], start=True, stop=False)
            nc.tensor.matmul(ps1[:], c["wbb1h"], x_fm[:, TN:2 * TN], start=False, stop=False)
            nc.tensor.matmul(ps1[:], c["wbb1l"], x_fm[:, 0:TN], start=False, stop=True)
            h1 = p_w.tile([EMB, TN], F32, tag="h1", bufs=3, name=f"h1_{it}")
            nc.scalar.activation(h1[:], ps1[:], AF.Gelu, bias=c["b1"])

            yield  # ==== P1: backbone fc2 (f32) + gelu ====
            ps2 = ps_bb.tile([EMB, TN], F32, tag="bb", name=f"ps2_{it}")
            nc.tensor.matmul(ps2[:], c["wbb2"], h1[:], start=True, stop=True)
            h2s = p_w.tile([128, TN], F32, tag="h2s", bufs=5, name=f"h2s_{it}")
            nc.scalar.activation(h2s[0:EMB, :], ps2[:], AF.Gelu, bias=c["b2"])

            yield  # ==== P2: h2^2 + bb-LN stats (swap matmuls) ====
            if kn["h2sq"] == "a":
                nc.scalar.activation(h2s[EMB:128, :], h2s[0:EMB, :], AF.Square)
            else:
                eng(kn["h2sq"]).tensor_tensor(h2s[EMB:128, :], h2s[0:EMB, :],
                                              h2s[0:EMB, :], op=ALU.mult)
            psA = ps_sm.tile([128, NCH, 2], F32, tag="sm", name=f"psA_{it}")
            for b in range(NCH):
                nc.tensor.matmul(psA[:, b, :], h2s[:, 128 * b:128 * (b + 1)],
                                 c["stat2"], start=True, stop=True)
            sA = p_sc.tile([128, NCH, 2], F32, tag="sA", bufs=3, name=f"sA_{it}")
            if kn.get("sa", "v") == "a":
                nc.scalar.copy(sA[:], psA[:])
            else:
                nc.vector.tensor_copy(sA[:], psA[:])

            yield  # ==== P3: pass A (bb LN scalars, batch-major) ====
            tmpA = p_sc.tile([128, NCH], F32, tag="tmpA", bufs=3, name=f"tmpA_{it}")
            nc.vector.tensor_tensor(tmpA[:], sA[:, :, 0], sA[:, :, 0], op=ALU.mult)
            vA = p_sc.tile([128, NCH], F32, tag="vA", bufs=3, name=f"vA_{it}")
            nc.vector.scalar_tensor_tensor(vA[:], sA[:, :, 1], EPS_LN, tmpA[:],
                                           op0=ALU.add, op1=ALU.subtract)
            backA = p_sc.tile([128, NCH, 2], F32, tag="backA", bufs=4, name=f"backA_{it}")
            rsA = backA[:, :, 0]
            if kn.get("rsa", "n") == "s":
                sqA = p_sc.tile([128, NCH], F32, tag="sqA", bufs=3, name=f"sqA_{it}")
                nc.scalar.activation(sqA[:], vA[:], AF.Sqrt)
                nc.vector.reciprocal(rsA, sqA[:])
            else:
                _newton_rsqrt(nc, p_sc, vA[:], rsA, [128, NCH], f"nA_{it}", niter=2)
            nc.vector.tensor_tensor(backA[:, :, 1], rsA, sA[:, :, 0], op=ALU.mult)
            backAb = p_sc.tile([128, NCH, 3], BF16, tag="backAb", bufs=3, name=f"backAb_{it}")
            nc.vector.tensor_copy(backAb[:, :, 0:2], backA[:])
            nc.gpsimd.memset(backAb[:, :, 2], 1.0)

            yield  # ==== P4: gate A matmul (f32) + gprod; rs/p transpose + stf ====
            psU0 = ps_bb.tile([128, TN], F32, tag="bb", name=f"psU0_{it}")
            nc.tensor.matmul(psU0[:], c["wgU0"], h2s[0:EMB, :], start=True, stop=True)
            gprod = p_w.tile([128, TN], F32, tag="gprod", bufs=4, name=f"gprod_{it}")
            nc.vector.tensor_tensor(gprod[:], psU0[:], u_fm[:], op=ALU.mult)

            psBA = ps_sm.tile([3, TN], BF16, tag="sm", name=f"psBA_{it}")
            for b in range(NCH):
                nc.tensor.transpose(psBA[:, 128 * b:128 * (b + 1)],
                                    backAb[:, b, :], identb)
            stf = p_sc.tile([3, TN], BF16, tag="stf", bufs=3, name=f"stf_{it}")
            if kn["stf"] == "a":
                nc.scalar.copy(stf[:], psBA[:])
            else:
                nc.vector.tensor_copy(stf[:], psBA[:])

            yield  # ==== P5: gate seg-sum (swap) + g1t; stp broadcast + t1h/hb ====
            A_bm = ps_sm.tile([128, NCH, E], F32, tag="sm", name=f"Abm_{it}")
            for b in range(NCH):
                nc.tensor.matmul(A_bm[:, b, :], gprod[:, 128 * b:128 * (b + 1)],
                                 c["gsum"], start=True, stop=True)
            g1t = p_sc.tile([128, NCH, E], F32, tag="g1t", bufs=3, name=f"g1t_{it}")
            nc.vector.tensor_tensor(g1t[:], A_bm[:], _bc(backA[:, :, 0:1], E),
                                    op=ALU.mult)

            stp = ps_bb.tile([128, TN], F32, tag="bb", name=f"stp_{it}")
            nc.tensor.matmul(stp[:], c["stlb"], stf[:], start=True, stop=True)
            hb = p_w.tile([EMB + 1, TN], BF16, tag="hb", bufs=4, name=f"hb_{it}")
            t1h = p_w.tile([EMB, TN], BF16, tag="t1h", bufs=3, name=f"t1h_{it}")
            nc.vector.tensor_tensor(t1h[:], h2s[0:EMB, :], stp[0:EMB, :], op=ALU.mult)
            nc.vector.tensor_tensor(hb[0:EMB, :], t1h[:], stp[EMB:128, :],
                                    op=ALU.subtract)
            nc.gpsimd.memset(hb[EMB:EMB + 1, :], 1.0)

            yield  # ==== P6: gate g = g1t - p*B + D ====
            g2t = p_sc.tile([128, NCH, E], F32, tag="g2t", bufs=3, name=f"g2t_{it}")
            eng(kn["gcp"]).tensor_tensor(g2t[:], BT_t, _bc(backA[:, :, 1:2], E),
                                         op=ALU.mult)
            g3t = p_sc.tile([128, NCH, E], F32, tag="g3t", bufs=3, name=f"g3t_{it}")
            eng(kn["gcp"]).tensor_tensor(g3t[:], g1t[:], g2t[:], op=ALU.subtract)
            gcp = p_sc.tile([128, NCH, E], F32, tag="gcp", bufs=5, name=f"gcp_{it}")
            eng(kn["gcp"]).tensor_tensor(gcp[:], g3t[:], DT_t, op=ALU.add)

            yield  # ==== P7: experts fc1 + gelu + z^2 ; top-2 gate + cst term ====
            z_sb = []
            for p in range(8):
                zq = ps_z.tile([128, TN], F32, tag="z", name=f"zq_{it}_{p}")
                nc.tensor.matmul(zq[:], c["we1b"][:, p, :], hb[:], start=True, stop=True)
                z = p_z.tile([128, TN], BF16, tag="z_sb", bufs=bu["zsb"], name=f"z_{it}_{p}")
                nc.scalar.activation(z[:], zq[:], AF.Gelu)
                z_sb.append(z)
            z2_sb = []
            na = int(kn.get("z2act", 0))
            np_ = int(kn["z2pool"])
            for p in range(8):
                z2 = p_z2.tile([128, TN], BF16, tag="z2_sb", bufs=bu["z2sb"],
                               name=f"z2_{it}_{p}")
                if p < na:
                    nc.scalar.activation(z2[:], z_sb[p][:], AF.Square)
                else:
                    e2 = nc.gpsimd if p < na + np_ else nc.vector
                    e2.tensor_tensor(z2[:], z_sb[p][:], z_sb[p][:], op=ALU.mult)
                z2_sb.append(z2)

            # top-2 selection (from gcp, P6) and w weights
            vm8 = p_sc.tile([128, NCH, 8], F32, tag="vm8", bufs=3, name=f"vm8_{it}")
            for ch in range(NCH):
                nc.vector.max(vm8[:, ch, :], gcp[:, ch, :])
            dg = p_sc.tile([128, NCH], F32, tag="dg", bufs=3, name=f"dg_{it}")
            nc.vector.tensor_tensor(dg[:], vm8[:, :, 0], vm8[:, :, 1], op=ALU.subtract)
            th = p_sc.tile([128, NCH], F32, tag="th", bufs=3, name=f"th_{it}")
            nc.scalar.activation(th[:], dg[:], AF.Tanh, scale=0.5)
            w12 = p_sc.tile([128, NCH, 2], F32, tag="w12", bufs=3, name=f"w12_{it}")
            nc.vector.tensor_scalar(w12[:, :, 0], th[:], 0.5, 0.5, op0=ALU.mult, op1=ALU.add)
            nc.vector.tensor_scalar(w12[:, :, 1], th[:], -0.5, 0.5, op0=ALU.mult, op1=ALU.add)
            is1 = p_sc.tile([128, NCH, E], F32, tag="is1", bufs=3, name=f"is1_{it}")
            nc.vector.tensor_tensor(is1[:], gcp[:], _bc(vm8[:, :, 0:1], E),
                                    op=ALU.is_equal)
            is2 = p_sc.tile([128, NCH, E], F32, tag="is2", bufs=3, name=f"is2_{it}")
            nc.vector.tensor_tensor(is2[:], gcp[:], _bc(vm8[:, :, 1:2], E),
                                    op=ALU.is_equal)
            w1t = p_sc.tile([128, NCH, E], F32, tag="w1t", bufs=3, name=f"w1t_{it}")
            eng(kn["isw"]).tensor_tensor(w1t[:], is1[:], _bc(w12[:, :, 0:1], E),
                                         op=ALU.mult)
            w2t = p_sc.tile([128, NCH, E], F32, tag="w2t", bufs=3, name=f"w2t_{it}")
            eng(kn["isw"]).tensor_tensor(w2t[:], is2[:], _bc(w12[:, :, 1:2], E),
                                         op=ALU.mult)
            wsum = p_sc.tile([128, NCH, E], F32, tag="wsum", bufs=4, name=f"wsum_{it}")
            eng(kn["w32"]).tensor_tensor(wsum[:], w1t[:], w2t[:], op=ALU.add)
            # cst term: sum_e w_e * cst[e,c] via transpose + tiny swap-matmul
            wv16 = p_sc.tile([128, NCH, E], BF16, tag="wv16", bufs=3, name=f"wv16_{it}")
            nc.vector.tensor_copy(wv16[:], wsum[:])
            psWc = ps_sm.tile([E, TN], BF16, tag="sm", name=f"psWc_{it}")
            for b in range(NCH):
                nc.tensor.transpose(psWc[:, 128 * b:128 * (b + 1)], wv16[:, b, :],
                                    identb)
            wT = p_sc.tile([E, TN], BF16, tag="wT", bufs=3, name=f"wT_{it}")
            if kn.get("wt", "v") == "a":
                nc.scalar.copy(wT[:], psWc[:])
            else:
                nc.vector.tensor_copy(wT[:], psWc[:])

            yield  # ==== P8: fc2 swap-matmuls + stats + rs + weighted products ====
            cstD = ps_sm.tile([128, NCH, NCLS], F32, tag="sm", name=f"cstD_{it}")
            for b in range(NCH):
                nc.tensor.matmul(cstD[:, b, :], wT[:, 128 * b:128 * (b + 1)],
                                 c["cstb"], start=True, stop=True)
            cstt = p_sc.tile([128, NCH, NCLS], F32, tag="cstt", bufs=4,
                             name=f"cstt_{it}")
            if kn.get("cstt", "v") == "a":
                nc.scalar.copy(cstt[:], cstD[:])
            else:
                nc.vector.tensor_copy(cstt[:], cstD[:])

            # P layout per half: [128, 2 blocks, 256] ; pair p at cols 24p..24p+24
            # cols: 0:10 cls_e0', 10:20 cls_e1', 20 mu_e0, 21 mu_e1, 22:24 m2
            # P psum is intra-phase scratch.
            statB = p_sc.tile([128, NCH, 8, 4], F32, tag="statB", bufs=3,
                              name=f"statB_{it}")
            Pts = []
            for h in range(2):
                Pt = ps_P.tile([128, 2, 256], F32, tag="P", name=f"P{h}_{it}")
                for bb in range(2):
                    b = 2 * h + bb
                    for p in range(8):
                        zsl = z_sb[p][:, 128 * b:128 * (b + 1)]
                        z2sl = z2_sb[p][:, 128 * b:128 * (b + 1)]
                        nc.tensor.matmul(Pt[:, bb, 24 * p:24 * p + 22], zsl,
                                         c["we2s"][:, p, :], start=True, stop=True)
                        nc.tensor.matmul(Pt[:, bb, 24 * p + 22:24 * p + 24], z2sl,
                                         c["zw2"], start=True, stop=True)
                Pv = Pt[:, :, 0:192].rearrange("p b (e k) -> p b e k", e=8, k=24)
                if kn["statb"] == "a":
                    nc.scalar.copy(statB[:, 2 * h:2 * h + 2], Pv[:, :, :, 20:24])
                else:
                    nc.vector.tensor_copy(statB[:, 2 * h:2 * h + 2], Pv[:, :, :, 20:24])
                Pcv = Pv[:, :, :, 0:20].rearrange("p b e (q c) -> p b e q c", q=2, c=10)
                if int(kn.get("prps", 0)):
                    Pts.append(Pcv)
                else:
                    pc = p_w.tile([128, 2, 8, 2, 10], BF16, tag="pcls", bufs=8,
                                  name=f"pc_{it}_{h}")
                    if kn.get("pcls", "a") == "a":
                        nc.scalar.copy(pc[:], Pcv)
                    else:
                        nc.vector.tensor_copy(pc[:], Pcv)
                    Pts.append(pc)

            yield  # ==== P9: pass B (rs) + weighted products ====
            muB = statB[:, :, :, 0:2]
            m2B = statB[:, :, :, 2:4]
            tmpB = p_sc.tile([128, NCH, 8, 2], F32, tag="tmpB", bufs=3, name=f"tmpB_{it}")
            eng(kn["passb"]).tensor_tensor(tmpB[:], muB, muB, op=ALU.mult)
            vB = p_sc.tile([128, NCH, 8, 2], F32, tag="vB", bufs=3, name=f"vB_{it}")
            nc.vector.scalar_tensor_tensor(vB[:], m2B, EPS_LN, tmpB[:],
                                           op0=ALU.add, op1=ALU.subtract)
            rsB = p_sc.tile([128, NCH, 8, 2], F32, tag="rsB", bufs=3, name=f"rsB_{it}")
            if kn.get("rsb", "n") == "s":
                sqB = p_sc.tile([128, NCH, 8, 2], F32, tag="sqB", bufs=3,
                                name=f"sqB_{it}")
                nc.scalar.activation(sqB[:], vB[:], AF.Sqrt)
                nc.vector.reciprocal(rsB[:], sqB[:])
            else:
                _newton_rsqrt(nc, p_sc, vB[:], rsB[:], [128, NCH, 8, 2],
                              f"nB_{it}", niter=1)
            rsBf = rsB[:].rearrange("p c e q -> p c (e q)")
            wsb16 = p_sc.tile([128, NCH, E], BF16, tag="wsb16", bufs=3,
                              name=f"wsb16_{it}")
            nc.vector.tensor_tensor(wsb16[:], wsum[:], rsBf, op=ALU.mult)

            prods = []
            for h in range(2):
                wsv = wsb16[:, 2 * h:2 * h + 2, :].rearrange(
                    "p b (e q o) -> p b e q o", e=8, q=2, o=1)
                pr = p_w.tile([128, 2, 8, 2, 10], BF16, tag="pr", bufs=6,
                              name=f"pr_{it}_{h}")
                src_h = Pts[h] if int(kn.get("prps", 0)) else Pts[h][:]
                nc.vector.tensor_tensor(pr[:], src_h, _bc(wsv, 10), op=ALU.mult)
                prods.append(pr)

            yield  # ==== P10: tree reduce + cst add ====
            osb = p_out.tile([128, NCH, NCLS], F32, tag="osb", bufs=3, name=f"osb_{it}")
            for h in range(2):
                pr = prods[h]
                te = eng(kn["tree"]) if kn["tree"] != "a" else nc.vector
                ta = p_sc.tile([128, 2, 4, 2, 10], BF16, tag="ta", bufs=3,
                               name=f"ta_{it}_{h}")
                te.tensor_tensor(ta[:], pr[:, :, 0:4], pr[:, :, 4:8], op=ALU.add)
                tb = p_sc.tile([128, 2, 2, 2, 10], BF16, tag="tb", bufs=3,
                               name=f"tb_{it}_{h}")
                te.tensor_tensor(tb[:], ta[:, :, 0:2], ta[:, :, 2:4], op=ALU.add)
                td = p_sc.tile([128, 2, 2, 10], BF16, tag="td", bufs=3,
                               name=f"td_{it}_{h}")
                te.tensor_tensor(td[:], tb[:, :, 0], tb[:, :, 1], op=ALU.add)
                tf = p_sc.tile([128, 2, NCLS], BF16, tag="tf", bufs=3,
                               name=f"tf_{it}_{h}")
                te.tensor_tensor(tf[:], td[:, :, 0], td[:, :, 1], op=ALU.add)
                nc.vector.tensor_tensor(osb[:, 2 * h:2 * h + 2, :], tf[:],
                                        cstt[:, 2 * h:2 * h + 2, :], op=ALU.add)

            yield  # ==== P11: output DMA ====
            nc.sync.dma_start(d_out.ap()[it], osb[:])

        NPH = 12
        gens = {}
        for k in range(ntiles + NPH - 1):
            if k < ntiles:
                gens[k] = tile_body(k)
            for idx in sorted(gens):
                if next(gens[idx], StopIteration) is StopIteration:
                    del gens[idx]

    nc.compile()
    return nc


def _newton_rsqrt(nc, pool, v_ap, out_ap, shape, tag, niter=2, eng=None):
    """out = 1/sqrt(v) via quake seed + Newton iterations."""
    eng = eng or nc.vector
    r = pool.tile(shape, F32, tag=tag[:3] + "_r", name=tag + "_r")
    t = pool.tile(shape, F32, tag=tag[:3] + "_t", name=tag + "_t")
    eng.tensor_scalar(r[:].bitcast(I32), v_ap.bitcast(I32), 1, None,
                      op0=ALU.logical_shift_right)
    eng.tensor_scalar(r[:].bitcast(I32), r[:].bitcast(I32), -1, 0x5F3759DF,
                      op0=ALU.mult, op1=ALU.add)
    for i in range(niter):
        dst = out_ap if i == niter - 1 else r[:]
        eng.tensor_tensor(t[:], r[:], r[:], op=ALU.mult)
        eng.scalar_tensor_tensor(t[:], t[:], -0.5, v_ap, op0=ALU.mult, op1=ALU.mult)
        eng.scalar_tensor_tensor(dst, t[:], 1.5, r[:], op0=ALU.add, op1=ALU.mult)


# ---------------------------------------------------------------------------
# host-side weight prep
# ---------------------------------------------------------------------------
def prep_consts(inp):
    f = np.float32
    import ml_dtypes
    bf = ml_dtypes.bfloat16
    e_w1, e_b1 = np.asarray(inp["e_w1"], f), np.asarray(inp["e_b1"], f)
    e_g = np.asarray(inp["e_g"], np.float64)
    e_beta = np.asarray(inp["e_beta"], np.float64)
    e_w2, e_b2 = np.asarray(inp["e_w2"], np.float64), np.asarray(inp["e_b2"], np.float64)
    bb_g = np.asarray(inp["bb_g"], np.float64)
    bb_beta = np.asarray(inp["bb_beta"], np.float64)
    gU = np.asarray(inp["gU"], np.float64)

    vals32 = {}
    vals32["wbb2"] = np.asarray(inp["bb_w2"], f)
    vals32["b1"] = np.asarray(inp["bb_b1"], f).reshape(EMB, 1)
    vals32["b2"] = np.asarray(inp["bb_b2"], f).reshape(EMB, 1)
    st = np.zeros((128, 2), f)
    st[0:64, 0] = 1.0 / 64
    st[64:128, 1] = 1.0 / 64
    vals32["stat2"] = st
    wgU0 = np.zeros((EMB, 128), np.float64)
    for e in range(E):
        wgU0[:, e * RANK:(e + 1) * RANK] = gU[e] * bb_g[:, None]
    vals32["wgU0"] = wgU0.astype(f)
    gs = np.zeros((128, E), f)
    for e in range(E):
        gs[e * RANK:(e + 1) * RANK, e] = 1.0
    vals32["gsum"] = gs

    vals16 = {}
    vals16["identb"] = np.eye(128, dtype=f)
    # stp rows: [rs; p; 1] -> stp[0:64]=g*rs ; stp[64:128]=g*p - beta
    stl = np.zeros((3, 128), np.float64)
    stl[0, 0:64] = bb_g
    stl[1, 64:128] = bb_g
    stl[2, 64:128] = -bb_beta
    vals16["stlb"] = stl
    # we1 with bias row 64 (per pair: e0 cols 0:64, e1 cols 64:128)
    we1 = np.zeros((EMB + 1, 8, 128), f)
    for p in range(8):
        we1[0:EMB, p, 0:64] = e_w1[2 * p]
        we1[0:EMB, p, 64:128] = e_w1[2 * p + 1]
        we1[EMB, p, 0:64] = e_b1[2 * p]
        we1[EMB, p, 64:128] = e_b1[2 * p + 1]
    vals16["we1b"] = we1.reshape(EMB + 1, 1024)
    # fc2 swap weights: we2' = g*w2 - (g@w2)/64 ; mu cols 20,21
    gw2 = np.einsum("ed,edc->ec", e_g, e_w2)
    we2n = e_g[:, :, None] * e_w2 - gw2[:, None, :] / 64.0   # [E, 64, 10]
    we2 = np.zeros((128, 8, 22), np.float64)
    for p in range(8):
        e0, e1 = 2 * p, 2 * p + 1
        we2[0:64, p, 0:10] = we2n[e0]
        we2[64:128, p, 10:20] = we2n[e1]
        we2[0:64, p, 20] = 1.0 / 64
        we2[64:128, p, 21] = 1.0 / 64
    vals16["we2s"] = we2.reshape(128, 176)
    zw = np.zeros((128, 2), f)
    zw[0:64, 0] = 1.0 / 64
    zw[64:128, 1] = 1.0 / 64
    vals16["zw2"] = zw
    # xexp [32, 202]: rows 0:16 ws-expansion (0/1), rows 16:32 w->cst cols
    cst = np.einsum("ed,edc->ec", e_beta, e_w2) + e_b2
    vals16["cstb"] = cst

    w1 = np.asarray(inp["bb_w1"], np.float64)
    w1h = w1.astype(np.float16)
    w1l = (w1 - w1h.astype(np.float64)).astype(np.float16)
    ch16 = np.zeros((128, 2 * EMB), np.float16)
    ch16[0:IN_F, 0:EMB] = w1h
    ch16[0:IN_F, EMB:2 * EMB] = w1l

    cf32 = np.zeros((128, CF32_COLS), f)
    for name, (p, o, w) in CF32_OFF.items():
        cf32[0:p, o:o + w] = vals32[name]
    cb16 = np.zeros((128, CB16_COLS), bf)
    for name, (p, o, w) in CB16_OFF.items():
        cb16[0:p, o:o + w] = np.asarray(vals16[name], np.float64).astype(bf)
    return {"cf32": cf32, "cb16": cb16, "ch16": ch16}


def prep_user_tables(inp):
    """uV gather table [NUSERS,128] plus per-user gate tables B, D [NUSERS,E]."""
    gU = np.asarray(inp["gU"], np.float64)
    gV = np.asarray(inp["gV"], np.float64)
    gb = np.asarray(inp["gb"], np.float64)
    ut = np.asarray(inp["ut"], np.float64)
    bb_g = np.asarray(inp["bb_g"], np.float64)
    bb_beta = np.asarray(inp["bb_beta"], np.float64)
    wgU = np.zeros((EMB, 128), np.float64)
    for e in range(E):
        wgU[:, e * RANK:(e + 1) * RANK] = gU[e]
    uV = np.einsum("ud,edr->uer", ut, gV).reshape(NUSERS, 128)  # [u, e*8+r]
    cg = (bb_g @ wgU).reshape(E, RANK)       # wgU^T g
    cb = (bb_beta @ wgU).reshape(E, RANK)    # wgU^T beta
    uV3 = uV.reshape(NUSERS, E, RANK)
    Btab = np.einsum("er,uer->ue", cg, uV3)
    Dtab = np.einsum("er,uer->ue", cb, uV3) + gb[None, :]
    return uV.astype(np.float32), Btab.astype(np.float32), Dtab.astype(np.float32)


def shard_inputs(x, user_ids, inp, b_core):
    """x [B,80] -> per-core [nt,80,1024] fp16 hi|lo feature-major;
    uV gathered+transposed; B/D tables gathered batch-major."""
    ncores = x.shape[0] // b_core
    nt = b_core // TN
    xr = x.astype(np.float64)
    xh = xr.astype(np.float16)
    xl = (xr - xh.astype(np.float64)).astype(np.float16)
    xhs = xh.reshape(ncores, nt, TN, IN_F).transpose(0, 1, 3, 2)
    xls = xl.reshape(ncores, nt, TN, IN_F).transpose(0, 1, 3, 2)
    xs = np.ascontiguousarray(np.concatenate([xhs, xls], axis=3))  # [.., 80, 1024]
    uV, Btab, Dtab = prep_user_tables(inp)
    u = uV[user_ids]                                   # [B, 128]
    us = np.ascontiguousarray(
        u.reshape(ncores, nt, TN, 128).transpose(0, 1, 3, 2))
    # batch-major: sample s at (row=s%128, ch=s//128); B and D side by side
    BD = np.concatenate([Btab[user_ids], Dtab[user_ids]], axis=-1)  # [B, 2E]
    BDg = BD.reshape(ncores, nt, NCH, 128, 2 * E)
    BDt = np.ascontiguousarray(BDg.transpose(0, 1, 3, 2, 4))  # [.., 128, NCH, 2E]
    return xs, us, BDt


_CACHE = {}


def _get_program(b_core, mmdt="hybrid"):
    key = (b_core, mmdt)
    if key not in _CACHE:
        _CACHE[key] = build_program(b_core, mmdt)
    return _CACHE[key]


def build_in_maps(inputs):
    x = np.asarray(inputs["x"], np.float64).reshape(B, IN_F)
    uids = np.asarray(inputs["user_ids"]).astype(np.int64)
    cns = prep_consts({k: np.asarray(v) for k, v in inputs.items()})
    xs, us, BDt = shard_inputs(x, uids, inputs, B_CORE)
    in_maps = []
    for k in range(NCORES):
        m = dict(cns)
        m["x"] = xs[k]
        m["u"] = us[k]
        m["BD"] = BDt[k]
        in_maps.append(m)
    return in_maps


def kernel(**inputs):
    from concourse.bass_utils import run_bass_kernel_spmd
    nc = _get_program(B_CORE)
    in_maps = build_in_maps(inputs)
    res = run_bass_kernel_spmd(nc, in_maps, core_ids=list(range(NCORES)))
    nt = B_CORE // TN
    # out [nt, 128, NCH, NCLS]: sample = it*TN + ch*128 + row
    outs = []
    for r in res.results:
        o = r["out"].reshape(nt, 128, NCH, NCLS).transpose(0, 2, 1, 3)
        outs.append(o.reshape(B_CORE, NCLS))
    return np.concatenate(outs, axis=0).astype(np.float32)
